# revision 26
# baseline (speedup 1.0000x reference)
"""Trainium2 Bass kernel for nn_Discriminator (2-layer LSTM, B=512 T=100 H=1024).

Strategy: data-parallel over batch across 8 cores (B=64 per core).
Per core:
  - Layer-1 input projections collapse through the encoder:
    A1 = x @ (W_ih1 @ enc_W)^T, a [6400,34]@[34,4096] matmul (K=34)
    instead of enc=[6400,1024] then [6400,1024]@[1024,4096].
    W_combT (+ gate bias row) is built incrementally from W_ih1 row
    chunks, so no 16MB transposed copy of W_ih1 is ever materialized.
  - Recurrence weights / h state / A scratch are bf16: half the DMA
    and SBUF traffic and lower PE power (the chip power-throttles the
    PE duty cycle under sustained all-engine load, so energy saved
    anywhere buys PE clock).
  - Gate bias + A-scratch are pre-added into PSUM before the gate
    matmuls (start=False), so activations read PSUM immediately after
    the last matmul.
"""

import numpy as np

import concourse.bass as bass
import concourse.tile as tile
import concourse.mybir as mybir
from concourse import bacc
from concourse.bass_utils import run_bass_kernel_spmd
from concourse.masks import make_identity

F32 = mybir.dt.float32
F32R = mybir.dt.float32r
BF16 = mybir.dt.bfloat16
AF = mybir.ActivationFunctionType

N_CORES = 8
B, IN, H = 512, 34, 1024
G = 4 * H  # 4096
BPC = B // N_CORES  # 64 batch rows per core
P = 128
KC = H // P  # 8 contraction chunks

WDT = BF16   # A2 weight dtype
ADT = BF16   # A1/A2 scratch dtype
HDT = BF16   # h^T ring dtype (flush / A2 path)
FP8 = mybir.dt.float8e4  # recurrence matmul dtype (DoubleRow)
KPF = KC // 2  # fp8 k-pairs per contraction


def _build_weight_T(nc, w_dram, w_T, identr, wrow, wtr_ps):
    """Transpose w_dram [G, H] into resident SBUF tile w_T [128, KC, G]
    (w_T[p, k, j] = W[j, 128k + p]) via PE transposes; w_T may be bf16
    (cast happens in the PSUM->SBUF copy)."""
    n_row_tiles = w_dram.shape[0] // P  # 32
    for r in range(n_row_tiles):
        wt = wrow.tile([P, H], F32R, tag="wrow")
        nc.sync.dma_start(wt[:], w_dram[r * P:(r + 1) * P, :].bitcast(F32R))
        for c in range(KC):
            pt = wtr_ps.tile([P, P], F32R, tag="wtr")
            nc.tensor.transpose(pt[:], wt[:, c * P:(c + 1) * P], identr[:])
            # gpsimd cannot touch PSUM; alternate vector / scalar(Copy)
            if c % 2 == 0:
                nc.vector.tensor_copy(w_T[:, c, r * P:(r + 1) * P], pt[:])
            else:
                nc.scalar.activation(w_T[:, c, r * P:(r + 1) * P], pt[:], AF.Copy)


def _build_weight_T8(nc, w_dram, w_T8, identr, wrow, wtr_ps):
    """Like _build_weight_T but emits fp8 [128, KPF, 2, G] (k-pair packed
    for DoubleRow): w_T8[p, kp, e, j] = W[j, 128*(2*kp+e) + p]."""
    n_row_tiles = w_dram.shape[0] // P  # 32
    for r in range(n_row_tiles):
        wt = wrow.tile([P, H], F32R, tag="wrow")
        nc.sync.dma_start(wt[:], w_dram[r * P:(r + 1) * P, :].bitcast(F32R))
        for c in range(KC):
            pt = wtr_ps.tile([P, P], F32R, tag="wtr")
            nc.tensor.transpose(pt[:], wt[:, c * P:(c + 1) * P], identr[:])
            dst = w_T8[:, c // 2, c % 2, r * P:(r + 1) * P]
            if c % 2 == 0:
                nc.vector.tensor_copy(dst, pt[:])
            else:
                nc.scalar.activation(dst, pt[:], AF.Copy)


def _emit_A2_phase(nc, T, *, w_T, lhs_blk_fn, a_dram_flat, psum_a, ev_pool):
    """A2 = lhs @ W^T (no bias), lhs supplied per 256-column block by
    lhs_blk_fn (returns SBUF tile [128, KC, 256] = lhs^T block).
    Output rows are (t*BPC + b) flattened, written [T*BPC, G] bf16."""
    n_blocks = (T * BPC) // 256
    for mb in range(n_blocks):
        lhsT_blk = lhs_blk_fn(mb)
        for mt in range(2):
            row0 = mb * 256 + mt * P
            for np_ in range(4):  # pairs of 512-wide n chunks
                pts = [psum_a.tile([P, 512], F32, tag="pa", name=f"pa{j}")
                       for j in range(2)]
                for k in range(KC):
                    for j in range(2):
                        n = np_ * 2 + j
                        nc.tensor.matmul(
                            pts[j][:],
                            lhsT_blk[:, k, mt * P:(mt + 1) * P],
                            w_T[:, k, n * 512:(n + 1) * 512],
                            start=(k == 0), stop=(k == KC - 1),
                        )
                for j in range(2):
                    n = np_ * 2 + j
                    ev = ev_pool.tile([P, 512], ADT, tag="aev")
                    if j == 0:
                        nc.vector.tensor_copy(ev[:], pts[j][:])
                    else:
                        nc.scalar.activation(ev[:], pts[j][:], AF.Copy)
                    nc.sync.dma_start(
                        a_dram_flat[row0:row0 + P, n * 512:(n + 1) * 512], ev[:]
                    )


def _emit_recurrence(nc, T, *, a_dram, bias128, w_T8, hT8, hTb, c_t, a_pool,
                     gact_pool, hpool, psum_g, psum_tr, misc_pool,
                     h1T_dram=None, dec=None, out_ap=None, ident=None,
                     identh=None, a2_fuse=None):
    """T sequential LSTM-cell steps for one layer.

    hT8: fp8 ring [128, KPF, 2, 8, BPC] (k-pair packed, DoubleRow lhsT);
    step t writes slot t%8, reads slot (t-1)%8.  hTb: optional bf16 ring
    [128, KC, 8, BPC] kept in parallel for the h1T flush / A2 path.
    c_t: [BPC, H] fp32 persistent cell state.
    Gate preactivation = PSUM preload (a_t + bias) + h @ W_hh^T (fp8
    DoubleRow: 4 matmuls per 512-wide gate half).
    Weight gate order along G: i, f, g, o.
    """
    a_tiles = {}
    pre_pg = {}

    def load_a(t):
        a1t = a_pool.tile([BPC, G], ADT, tag="a1t", name=f"a1t_{t % 4}")
        nc.sync.dma_start(a1t[:], a_dram[t])
        a_tiles[t] = a1t

    def preload(t, g_idx):
        # preload PSUM with a_t + bias for this gate (vector engine;
        # gpsimd cannot write PSUM)
        pg = psum_g.tile([BPC, H], F32, tag="pg", name=f"pg{g_idx}")
        sl_g = slice(g_idx * H, (g_idx + 1) * H)
        nc.vector.tensor_add(pg[:], a_tiles[t][:, sl_g], bias128[:BPC, sl_g])
        pre_pg[(t, g_idx)] = pg

    def emit_a2_block(b):
        # A2 rows [b*256 : b*256+256] = h1 of steps 4b..4b+3 (ring slots
        # (4b)%8..+3), M=128 via 2 slots per m-tile; lands in the PE's
        # tail-idle window of the current step
        w_T2, a2_flat, psum_a2, ev2 = a2_fuse
        s0 = (4 * b) % 8
        for mt in range(2):
            row0 = b * 256 + mt * P
            for n in range(8):
                pa = psum_a2.tile([P, 512], F32, tag="pa")
                for k in range(KC):
                    nc.tensor.matmul(
                        pa[:], hTb[:, k, s0 + 2 * mt:s0 + 2 * mt + 2, :],
                        w_T2[:, k, n * 512:(n + 1) * 512],
                        start=(k == 0), stop=(k == KC - 1))
                ev = ev2.tile([P, 512], ADT, tag="aev2")
                if n % 2 == 0:
                    nc.vector.tensor_copy(ev[:], pa[:])
                else:
                    nc.scalar.activation(ev[:], pa[:], AF.Copy)
                nc.sync.dma_start(
                    a2_flat[row0:row0 + P, n * 512:(n + 1) * 512], ev[:])

    load_a(0)
    preload(0, 0)
    preload(0, 2)

    for t in range(T):
        if a2_fuse is not None and t >= 4 and t % 4 == 0:
            emit_a2_block((t - 4) // 4)
        s_r = (t + 7) % 8
        s_w = t % 8
        if t + 1 < T:
            load_a(t + 1)
        a1t = a_tiles[t]

        acts = {}

        def mm_gate(g_idx, pg, n2_outer=False):
            loops = ([(n2, kp) for n2 in (1, 0) for kp in range(KPF)] if n2_outer
                     else [(n2, kp) for kp in range(KPF) for n2 in range(2)])
            for n2, kp in loops:
                n = g_idx * 2 + n2
                nc.tensor.matmul(
                    pg[:, n2 * 512:(n2 + 1) * 512],
                    hT8[:, kp, :, s_r, :],
                    w_T8[:, kp, :, n * 512:(n + 1) * 512],
                    start=False, stop=(kp == KPF - 1),
                    perf_mode=mybir.MatmulPerfMode.DoubleRow,
                    skip_group_check=True,
                )

        def do_gate(g_idx, func, tag):
            pg = pre_pg.pop((t, g_idx))
            mm_gate(g_idx, pg)
            at = gact_pool.tile([BPC, H], HDT, tag="gact", name=tag)
            nc.scalar.activation(at[:], pg[:], func)
            acts[g_idx] = at

        HF = 512  # half of H, processed separately to shorten the serial tail
        do_gate(0, AF.Sigmoid, "act_i")        # input gate
        do_gate(2, AF.Tanh, "act_g")           # candidate
        tmp = gact_pool.tile([BPC, H], HDT, tag="gact", name="tmp")
        nc.vector.tensor_mul(tmp[:], acts[0][:], acts[2][:])

        # o/f preloads emitted BEFORE the c-update chain so they don't queue
        # behind it on the vector engine (their banks recycle pg_i / pg_g,
        # whose last readers ran early)
        preload(t, 3)
        preload(t, 1)
        pg_f = pre_pg.pop((t, 1))
        mm_gate(1, pg_f, n2_outer=True)
        act_f = gact_pool.tile([BPC, H], HDT, tag="gact", name="act_f")
        tanh_c = gact_pool.tile([BPC, H], HDT, tag="gact", name="tanh_c")
        for hh in (1, 0):
            # halves split across vector / gpsimd so the two chains run
            # concurrently (all operands SBUF)
            sl = slice(hh * HF, (hh + 1) * HF)
            eng = nc.vector if hh else nc.gpsimd
            nc.scalar.activation(act_f[:, sl], pg_f[:, sl], AF.Sigmoid)
            eng.tensor_mul(c_t[:, sl], c_t[:, sl], act_f[:, sl])
            eng.tensor_add(c_t[:, sl], c_t[:, sl], tmp[:, sl])
            nc.scalar.activation(tanh_c[:, sl], c_t[:, sl], AF.Tanh)

        # output gate + h + h^T, in halves so hT chunks stream out early
        pg_o = pre_pg.pop((t, 3))
        mm_gate(3, pg_o, n2_outer=True)
        act_o = gact_pool.tile([BPC, H], HDT, tag="gact", name="act_o")
        h_t = hpool.tile([BPC, H], HDT, tag="h_t")
        for hh in (1, 0):
            sl = slice(hh * HF, (hh + 1) * HF)
            eng = nc.vector if hh else nc.gpsimd
            nc.scalar.activation(act_o[:, sl], pg_o[:, sl], AF.Sigmoid)
            eng.tensor_mul(h_t[:, sl], act_o[:, sl], tanh_c[:, sl])

        # next step's i/g preloads ride in this step's tail so the PE never
        # waits on a preload at the step boundary
        if t + 1 < T:
            preload(t + 1, 0)
            preload(t + 1, 2)

        # reversed: chunk 0 (needed first by next step) lands last, so the
        # scheduler cannot interleave next-step matmuls with the transposes
        for k in range(KC - 1, -1, -1):
            pt = psum_tr.tile([P, BPC], HDT, tag="htr")
            nc.tensor.transpose(pt[:], h_t[:, k * P:(k + 1) * P], identh[:BPC, :BPC])
            if hTb is not None:
                # bf16 ring for flush/A2; fp8 ring fed from it via gpsimd
                nc.vector.tensor_copy(hTb[:, k, s_w, :], pt[:])
                nc.gpsimd.tensor_copy(hT8[:, k // 2, k % 2, s_w, :],
                                      hTb[:, k, s_w, :])
            else:
                nc.vector.tensor_copy(hT8[:, k // 2, k % 2, s_w, :], pt[:])
                if dec is not None and t == T - 1:
                    # bf16 copy of the final h2^T for a full-precision decode
                    nc.scalar.activation(dec[3][:, k, :], pt[:], AF.Copy)

        if h1T_dram is not None and (s_w == 7 or t == T - 1):
            # flush the ring (contiguous runs per partition)
            nslots = s_w + 1
            col0 = (t // 8) * 8 * BPC
            nc.sync.dma_start(
                h1T_dram.rearrange("(c p) n -> p c n", p=P)[:, :, col0:col0 + nslots * BPC],
                hTb[:, :, 0:nslots, :],
            )

        if dec is not None and t == T - 1:
            decWT, decb_sb, ones_bpc, hT_last = dec
            pd = psum_g.tile([1, BPC], F32, tag="pg", name="pdec")
            for k in range(KC):
                nc.tensor.matmul(pd[:], decWT[:, k:k + 1],
                                 hT_last[:, k, :],
                                 start=(k == 0), stop=False)
            nc.tensor.matmul(pd[:], decb_sb[:], ones_bpc[:],
                             start=False, stop=True)
            osb = misc_pool.tile([1, BPC], F32, tag="osb")
            nc.vector.tensor_copy(osb[:], pd[:])
            nc.sync.dma_start(out_ap.rearrange("b o -> o b"), osb[:])

    if a2_fuse is not None:
        emit_a2_block(T // 4 - 1)


def build(T=100):
    nc = bacc.Bacc("TRN2", target_bir_lowering=False, debug=False,
                   num_devices=N_CORES)

    x = nc.dram_tensor("x", [BPC, T, IN], F32, kind="ExternalInput").ap()
    enc_W = nc.dram_tensor("enc_W", [H, IN], F32, kind="ExternalInput").ap()
    enc_b = nc.dram_tensor("enc_b", [H], F32, kind="ExternalInput").ap()
    W_ih1 = nc.dram_tensor("W_ih1", [G, H], F32, kind="ExternalInput").ap()
    W_hh1 = nc.dram_tensor("W_hh1", [G, H], F32, kind="ExternalInput").ap()
    b_ih1 = nc.dram_tensor("b_ih1", [G], F32, kind="ExternalInput").ap()
    b_hh1 = nc.dram_tensor("b_hh1", [G], F32, kind="ExternalInput").ap()
    W_ih2 = nc.dram_tensor("W_ih2", [G, H], F32, kind="ExternalInput").ap()
    W_hh2 = nc.dram_tensor("W_hh2", [G, H], F32, kind="ExternalInput").ap()
    b_ih2 = nc.dram_tensor("b_ih2", [G], F32, kind="ExternalInput").ap()
    b_hh2 = nc.dram_tensor("b_hh2", [G], F32, kind="ExternalInput").ap()
    dec_W = nc.dram_tensor("dec_W", [1, H], F32, kind="ExternalInput").ap()
    dec_b = nc.dram_tensor("dec_b", [1], F32, kind="ExternalInput").ap()
    out = nc.dram_tensor("out", [BPC, 1], F32, kind="ExternalOutput").ap()

    A1 = nc.dram_tensor("A1_scratch", [T, BPC, G], ADT).ap()
    A2 = nc.dram_tensor("A2_scratch", [T, BPC, G], ADT).ap()
    h1T = nc.dram_tensor("h1T_scratch", [H, T * BPC], HDT).ap()
    A1_flat = A1.rearrange("t b g -> (t b) g")
    A2_flat = A2.rearrange("t b g -> (t b) g")

    with tile.TileContext(nc) as tc:
        with tc.tile_pool(name="persist", bufs=1) as persist, \
             tc.tile_pool(name="state", bufs=1) as state, \
             tc.tile_pool(name="misc", bufs=1) as misc:

            ident = persist.tile([P, P], F32, tag="ident")
            make_identity(nc, ident[:])
            identr = persist.tile([P, P], F32R, tag="identr")
            nc.vector.tensor_copy(identr[:], ident[:])
            identh = persist.tile([P, P], HDT, tag="identh")
            nc.vector.tensor_copy(identh[:], ident[:])
            ones1 = persist.tile([1, P], F32R, tag="ones1")
            nc.gpsimd.memset(ones1[:].bitcast(F32), 1.0)

            # persistent layer-1 input-side operands
            W_combT = persist.tile([IN, G], F32R, tag="W_combT")
            bias128_1 = persist.tile([P, G], ADT, tag="bias128_1")
            bias128_2 = persist.tile([P, G], ADT, tag="bias128_2")

            def bcast_bias_row(brow, dst, ps_pool):
                """dst[p, n*512:(n+1)*512] = brow[0, n*512:...] for all p."""
                for n in range(8):
                    sl = slice(n * 512, (n + 1) * 512)
                    pb2 = ps_pool.tile([P, 512], F32, tag="pbb")
                    nc.tensor.matmul(pb2[:], ones1[:], brow[:, sl],
                                     start=True, stop=True)
                    nc.vector.tensor_copy(dst[:, sl], pb2[:])

            # ============ Phase E: xT [IN, T*BPC] ============
            with tc.tile_pool(name="xt_pool", bufs=1) as xt_pool:
                xT = xt_pool.tile([IN, T * BPC], F32R, tag="xT")
                with nc.named_scope("phaseE"):
                    with tc.tile_pool(name="e_sb", bufs=3) as e_sb, \
                         tc.tile_pool(name="e_ps", bufs=3, space="PSUM") as e_ps:
                        xr = x.rearrange("b t f -> t b f")
                        for m in range((T * BPC) // P):
                            xt_ = e_sb.tile([P, IN], F32R, tag="xtile")
                            nc.sync.dma_start(xt_[:BPC, :], xr[2 * m].bitcast(F32R))
                            nc.sync.dma_start(xt_[BPC:, :], xr[2 * m + 1].bitcast(F32R))
                            pt = e_ps.tile([IN, P], F32R, tag="xtr")
                            nc.tensor.transpose(pt[:], xt_[:], identr[:])
                            nc.vector.tensor_copy(xT[:, m * P:(m + 1) * P], pt[:])

                # ============ W_combT build (incremental, no 16MB W_ih1^T) ====
                # encwb [128, KC, IN+1]: cols 0..IN-1 = enc_W chunk rows,
                # col IN = enc_b chunk. A single lhsT gives both W_combT rows
                # and the enc_b @ W_ih1^T bias row in one PSUM pass.
                with nc.named_scope("build_Wcomb"):
                    with tc.tile_pool(name="wc_sb", bufs=1) as wc_sb, \
                         tc.tile_pool(name="wc_row", bufs=3) as wc_row, \
                         tc.tile_pool(name="wc_st", bufs=2) as wc_st, \
                         tc.tile_pool(name="wc_ps", bufs=2, space="PSUM") as wc_ps, \
                         tc.tile_pool(name="wc_ps2", bufs=1, space="PSUM") as wc_ps2:
                        encwb = wc_sb.tile([P, KC, IN], F32R, tag="encwb")
                        nc.sync.dma_start(
                            encwb[:],
                            enc_W.rearrange("(c p) f -> p c f", p=P).bitcast(F32R))
                        encb_k = wc_sb.tile([P, KC], F32R, tag="encb_k")
                        nc.sync.dma_start(
                            encb_k[:],
                            enc_b.rearrange("(c p) -> p c", p=P).bitcast(F32R))
                        brow1 = wc_sb.tile([1, G], F32R, tag="brow1")
                        bi1 = wc_sb.tile([1, G], F32, tag="bi1")
                        nc.sync.dma_start(bi1[:], b_ih1[None, :])
                        bh1 = wc_sb.tile([1, G], F32, tag="bh1")
                        nc.sync.dma_start(bh1[:], b_hh1[None, :])
                        # groups of 4 row-chunks = 512 G columns
                        for grp in range(G // 512):
                            wstage = wc_st.tile([P, KC, 512], F32R, tag="wstage")
                            for rr in range(4):
                                r = grp * 4 + rr
                                wt = wc_row.tile([P, H], F32R, tag="wcrow")
                                nc.sync.dma_start(
                                    wt[:], W_ih1[r * P:(r + 1) * P, :].bitcast(F32R))
                                for c in range(KC):
                                    ptr = wc_ps.tile([P, P], F32R, tag="wctr")
                                    nc.tensor.transpose(
                                        ptr[:], wt[:, c * P:(c + 1) * P], identr[:])
                                    if c % 2 == 0:
                                        nc.vector.tensor_copy(
                                            wstage[:, c, rr * P:(rr + 1) * P], ptr[:])
                                    else:
                                        nc.scalar.activation(
                                            wstage[:, c, rr * P:(rr + 1) * P],
                                            ptr[:], AF.Copy)
                            pb = wc_ps2.tile([IN, 512], F32, tag="wcpb")
                            pbias = wc_ps2.tile([1, 512], F32, tag="wcpbias")
                            for k in range(KC):
                                nc.tensor.matmul(pb[:], encwb[:, k, :],
                                                 wstage[:, k, :],
                                                 start=(k == 0), stop=(k == KC - 1))
                            for k in range(KC):
                                nc.tensor.matmul(pbias[:], encb_k[:, k:k + 1],
                                                 wstage[:, k, :],
                                                 start=(k == 0), stop=(k == KC - 1))
                            sl = slice(grp * 512, (grp + 1) * 512)
                            nc.vector.tensor_copy(W_combT[:, sl], pb[:])
                            # bias row = enc_b@W^T + b_ih1 + b_hh1
                            nc.vector.tensor_add(brow1[:, sl], pbias[:],
                                                 bi1[:, sl])
                            nc.gpsimd.tensor_add(brow1[:, sl], brow1[:, sl],
                                                 bh1[:, sl])
                        bcast_bias_row(brow1, bias128_1, wc_ps2)

                # ============ Phase A1: A1 = x @ W_comb^T (K=34) ============
                with nc.named_scope("phaseA1"):
                    with tc.tile_pool(name="a1_ps", bufs=4, space="PSUM") as a_ps, \
                         tc.tile_pool(name="a1_ev", bufs=4) as a_ev:
                        for m in range((T * BPC) // P):
                            for n in range(8):
                                pa = a_ps.tile([P, 512], F32, tag="pa1")
                                nc.tensor.matmul(
                                    pa[:], xT[:, m * P:(m + 1) * P],
                                    W_combT[:, n * 512:(n + 1) * 512],
                                    start=True, stop=True)
                                ev = a_ev.tile([P, 512], ADT, tag="a1ev")
                                if n % 2 == 0:
                                    nc.vector.tensor_copy(ev[:], pa[:])
                                else:
                                    nc.scalar.activation(ev[:], pa[:], AF.Copy)
                                nc.sync.dma_start(
                                    A1_flat[m * P:(m + 1) * P,
                                            n * 512:(n + 1) * 512], ev[:])

            # ===== W_hh1^T + W_ih2^T builds + Phase R1 (A2 fused in) =====
            with tc.tile_pool(name="wpool", bufs=1) as wpool:
                w_T1 = wpool.tile([P, KPF, 2, G], FP8, tag="W8")
                with nc.named_scope("build_Whh1T"):
                    with tc.tile_pool(name="wrow1", bufs=3) as wrow, \
                         tc.tile_pool(name="wtr_ps1", bufs=3, space="PSUM") as wtr_ps:
                        _build_weight_T8(nc, W_hh1, w_T1, identr, wrow, wtr_ps)
                w_T2 = wpool.tile([P, KC, G], WDT, tag="W")
                with nc.named_scope("build_Wih2T"):
                    with tc.tile_pool(name="wrow2", bufs=3) as wrow, \
                         tc.tile_pool(name="wtr_ps2", bufs=3, space="PSUM") as wtr_ps:
                        _build_weight_T(nc, W_ih2, w_T2, identr, wrow, wtr_ps)

                hT8 = state.tile([P, KPF, 2, 8, BPC], FP8, tag="hT8_ring")
                hTb = state.tile([P, KC, 8, BPC], HDT, tag="hTb_ring")
                c_t = state.tile([BPC, H], F32, tag="c_t")
                nc.gpsimd.memset(hT8[:].bitcast(mybir.dt.uint8), 0.0)
                nc.gpsimd.memset(hTb[:].bitcast(mybir.dt.uint16), 0.0)
                nc.gpsimd.memset(c_t[:], 0.0)

                with nc.named_scope("phaseR1"):
                    with tc.tile_pool(name="r1_a", bufs=2) as a_pool, \
                         tc.tile_pool(name="r1_g", bufs=4) as gact_pool, \
                         tc.tile_pool(name="r1_h", bufs=1) as hpool, \
                         tc.tile_pool(name="r1_ev2", bufs=2) as ev2_pool, \
                         tc.tile_pool(name="r1_pg", bufs=2, space="PSUM") as psum_g, \
                         tc.tile_pool(name="r1_ptr", bufs=2, space="PSUM") as psum_tr, \
                         tc.tile_pool(name="r1_pa2", bufs=2, space="PSUM") as psum_a2:
                        _emit_recurrence(nc, T, a_dram=A1, bias128=bias128_1,
                                         w_T8=w_T1, hT8=hT8, hTb=hTb, c_t=c_t,
                                         a_pool=a_pool, gact_pool=gact_pool,
                                         hpool=hpool, psum_g=psum_g,
                                         psum_tr=psum_tr, misc_pool=misc,
                                         h1T_dram=None, ident=ident,
                                         identh=identh,
                                         a2_fuse=(w_T2, A2_flat, psum_a2,
                                                  ev2_pool))

                # layer-2 bias row: b_ih2 + b_hh2 broadcast
                with tc.tile_pool(name="b2_sb", bufs=1) as b2_sb, \
                     tc.tile_pool(name="b2_ps", bufs=2, space="PSUM") as b2_ps:
                    bi2 = b2_sb.tile([1, G], F32, tag="bi2")
                    nc.sync.dma_start(bi2[:], b_ih2[None, :])
                    bh2 = b2_sb.tile([1, G], F32, tag="bh2")
                    nc.sync.dma_start(bh2[:], b_hh2[None, :])
                    brow2 = b2_sb.tile([1, G], F32R, tag="brow2")
                    nc.vector.tensor_add(brow2[:], bi2[:], bh2[:])
                    bcast_bias_row(brow2, bias128_2, b2_ps)

                # ============ W_hh2^T build + Phase R2 (+decode) ============
                w_T3 = wpool.tile([P, KPF, 2, G], FP8, tag="W8")
                with nc.named_scope("build_Whh2T"):
                    with tc.tile_pool(name="wrow3", bufs=3) as wrow, \
                         tc.tile_pool(name="wtr_ps3", bufs=3, space="PSUM") as wtr_ps:
                        _build_weight_T8(nc, W_hh2, w_T3, identr, wrow, wtr_ps)

                nc.gpsimd.memset(hT8[:].bitcast(mybir.dt.uint8), 0.0)
                nc.gpsimd.memset(c_t[:], 0.0)

                decWT_f = misc.tile([P, KC], F32, tag="decWT_f")
                nc.sync.dma_start(decWT_f[:], dec_W.rearrange("o (c p) -> p (c o)", p=P))
                decWT = misc.tile([P, KC], HDT, tag="decWT")
                nc.vector.tensor_copy(decWT[:], decWT_f[:])
                decb_f = misc.tile([1, 1], F32, tag="decb_f")
                nc.sync.dma_start(decb_f[:], dec_b[None, :])
                decb_sb = misc.tile([1, 1], HDT, tag="decb")
                nc.vector.tensor_copy(decb_sb[:], decb_f[:])
                ones_f = misc.tile([1, BPC], F32, tag="ones_f")
                nc.gpsimd.memset(ones_f[:], 1.0)
                ones_bpc = misc.tile([1, BPC], HDT, tag="ones_bpc")
                hT_last = misc.tile([P, KC, BPC], HDT, tag="hT_last")
                nc.vector.tensor_copy(ones_bpc[:], ones_f[:])

                with nc.named_scope("phaseR2"):
                    with tc.tile_pool(name="r2_a", bufs=2) as a_pool, \
                         tc.tile_pool(name="r2_g", bufs=4) as gact_pool, \
                         tc.tile_pool(name="r2_h", bufs=1) as hpool, \
                         tc.tile_pool(name="r2_pg", bufs=3, space="PSUM") as psum_g, \
                         tc.tile_pool(name="r2_ptr", bufs=2, space="PSUM") as psum_tr:
                        _emit_recurrence(nc, T, a_dram=A2, bias128=bias128_2,
                                         w_T8=w_T3, hT8=hT8, hTb=None, c_t=c_t,
                                         a_pool=a_pool, gact_pool=gact_pool,
                                         hpool=hpool, psum_g=psum_g,
                                         psum_tr=psum_tr, misc_pool=misc,
                                         h1T_dram=None,
                                         dec=(decWT, decb_sb, ones_bpc, hT_last),
                                         out_ap=out, ident=ident, identh=identh)

    nc.compile()
    return nc


_cached_nc = None
_cached_fn = None  # (jitted shard_map fn, in_names, out_names, out_shapes, zeros)


def _build_jitted(nc):
    """Same lowering as bass2jax.run_bass_via_pjrt, but the jitted
    executable is cached so repeat kernel() calls skip recompilation."""
    import jax
    from jax.sharding import Mesh, PartitionSpec
    from jax.experimental.shard_map import shard_map
    from concourse import bass2jax, mybir as _mybir

    bass2jax.install_neuronx_cc_hook()
    partition_name = nc.partition_id_tensor.name if nc.partition_id_tensor else None
    in_names, out_names, out_avals, zero_outs = [], [], [], []
    for alloc in nc.m.functions[0].allocations:
        if not isinstance(alloc, _mybir.MemoryLocationSet):
            continue
        name = alloc.memorylocations[0].name
        if alloc.kind == "ExternalInput":
            if name != partition_name:
                in_names.append(name)
        elif alloc.kind == "ExternalOutput":
            shape = tuple(alloc.tensor_shape)
            dtype = _mybir.dt.np(alloc.dtype)
            out_names.append(name)
            out_avals.append(jax.core.ShapedArray(shape, dtype))
            zero_outs.append(np.zeros(shape, dtype))
    n_params = len(in_names)
    n_outs = len(out_avals)
    all_in_names = list(in_names) + list(out_names)
    if partition_name is not None:
        all_in_names.append(partition_name)
    donate = tuple(range(n_params, n_params + n_outs))

    def _body(*args):
        operands = list(args)
        if partition_name is not None:
            operands.append(bass2jax.partition_id_tensor())
        outs = bass2jax._bass_exec_p.bind(
            *operands,
            out_avals=tuple(out_avals),
            in_names=tuple(all_in_names),
            out_names=tuple(out_names),
            lowering_input_output_aliases=(),
            sim_require_finite=True,
            sim_require_nnan=True,
            nc=nc,
        )
        return tuple(outs)

    devices = jax.devices()[:N_CORES]
    mesh = Mesh(np.asarray(devices), ("core",))
    in_specs = (PartitionSpec("core"),) * (n_params + n_outs)
    out_specs = (PartitionSpec("core"),) * n_outs
    fn = jax.jit(
        shard_map(_body, mesh=mesh, in_specs=in_specs, out_specs=out_specs,
                  check_rep=False),
        donate_argnums=donate, keep_unused=True,
    )
    out_shapes = [a.shape for a in out_avals]
    return fn, in_names, out_names, out_shapes, zero_outs


_dev_cache = {}  # name -> (digest, device_array)


def _to_device(name, arr):
    """Replicate-concat a weight to all cores and keep it on device across
    calls (keyed by content hash) so repeat kernel() calls only ship x."""
    import hashlib
    import jax
    d = hashlib.blake2b(arr.tobytes(), digest_size=16).digest()
    hit = _dev_cache.get(name)
    if hit is not None and hit[0] == d:
        return hit[1]
    conc = np.concatenate([arr] * N_CORES, axis=0)
    darr = jax.device_put(conc)
    _dev_cache[name] = (d, darr)
    return darr


def kernel(**inputs):
    global _cached_nc, _cached_fn
    if _cached_nc is None:
        _cached_nc = build(100)
        _cached_fn = _build_jitted(_cached_nc)
    fn, in_names, out_names, out_shapes, zero_outs = _cached_fn
    ins = {k: np.ascontiguousarray(np.asarray(v, dtype=np.float32))
           for k, v in inputs.items()}
    concat_in = []
    for name in in_names:
        if name == "x":
            concat_in.append(ins["x"])  # already [512, T, IN]; axis0 shards
        else:
            concat_in.append(_to_device(name, ins[name]))
    i = out_names.index("out")
    last_err = None
    for attempt in range(3):
        try:
            concat_zeros = [np.zeros((N_CORES * z.shape[0], *z.shape[1:]), z.dtype)
                            for z in zero_outs]
            out_arrs = fn(*concat_in, *concat_zeros)
            outp = np.asarray(out_arrs[i]).reshape(B, 1)
            return outp.astype(np.float32)
        except Exception as e:  # transient NRT_EXEC_UNIT_UNRECOVERABLE etc.
            last_err = e
            _dev_cache.clear()
            concat_in = []
            for name in in_names:
                if name == "x":
                    concat_in.append(ins["x"])
                else:
                    concat_in.append(_to_device(name, ins[name]))
    raise last_err


# revision 28
# speedup vs baseline: 1.0817x; 1.0817x over previous
"""Trainium2 Bass kernel for nn_Discriminator (2-layer LSTM, B=512 T=100 H=1024).

Strategy: data-parallel over batch across 8 cores (B=64 per core).
Per core:
  - Layer-1 input projections collapse through the encoder:
    A1 = x @ (W_ih1 @ enc_W)^T, a [6400,34]@[34,4096] matmul (K=34)
    instead of enc=[6400,1024] then [6400,1024]@[1024,4096].
    W_combT (+ gate bias row) is built incrementally from W_ih1 row
    chunks, so no 16MB transposed copy of W_ih1 is ever materialized.
  - Recurrence weights / h state / A scratch are bf16: half the DMA
    and SBUF traffic and lower PE power (the chip power-throttles the
    PE duty cycle under sustained all-engine load, so energy saved
    anywhere buys PE clock).
  - Gate bias + A-scratch are pre-added into PSUM before the gate
    matmuls (start=False), so activations read PSUM immediately after
    the last matmul.
"""

import numpy as np

import concourse.bass as bass
import concourse.tile as tile
import concourse.mybir as mybir
from concourse import bacc
from concourse.bass_utils import run_bass_kernel_spmd
from concourse.masks import make_identity

F32 = mybir.dt.float32
F32R = mybir.dt.float32r
BF16 = mybir.dt.bfloat16
AF = mybir.ActivationFunctionType

N_CORES = 8
B, IN, H = 512, 34, 1024
G = 4 * H  # 4096
BPC = B // N_CORES  # 64 batch rows per core
P = 128
KC = H // P  # 8 contraction chunks

WDT = BF16   # A2 weight dtype
ADT = BF16   # A1/A2 scratch dtype
HDT = BF16   # h^T ring dtype (flush / A2 path)
FP8 = mybir.dt.float8e4  # recurrence matmul dtype (DoubleRow)
KPF = KC // 2  # fp8 k-pairs per contraction


def _build_weight_T(nc, w_dram, w_T, identr, wrow, wtr_ps):
    """Transpose w_dram [G, H] into resident SBUF tile w_T [128, KC, G]
    (w_T[p, k, j] = W[j, 128k + p]) via PE transposes; w_T may be bf16
    (cast happens in the PSUM->SBUF copy)."""
    n_row_tiles = w_dram.shape[0] // P  # 32
    for r in range(n_row_tiles):
        wt = wrow.tile([P, H], F32R, tag="wrow")
        nc.sync.dma_start(wt[:], w_dram[r * P:(r + 1) * P, :].bitcast(F32R))
        for c in range(KC):
            pt = wtr_ps.tile([P, P], F32R, tag="wtr")
            nc.tensor.transpose(pt[:], wt[:, c * P:(c + 1) * P], identr[:])
            # gpsimd cannot touch PSUM; alternate vector / scalar(Copy)
            if c % 2 == 0:
                nc.vector.tensor_copy(w_T[:, c, r * P:(r + 1) * P], pt[:])
            else:
                nc.scalar.activation(w_T[:, c, r * P:(r + 1) * P], pt[:], AF.Copy)


def _build_weight_T8(nc, w_dram, w_T8, identr, wrow, wtr_ps):
    """Like _build_weight_T but emits fp8 [128, KPF, 2, G] (k-pair packed
    for DoubleRow): w_T8[p, kp, e, j] = W[j, 128*(2*kp+e) + p]."""
    n_row_tiles = w_dram.shape[0] // P  # 32
    for r in range(n_row_tiles):
        wt = wrow.tile([P, H], F32R, tag="wrow")
        nc.sync.dma_start(wt[:], w_dram[r * P:(r + 1) * P, :].bitcast(F32R))
        for c in range(KC):
            pt = wtr_ps.tile([P, P], F32R, tag="wtr")
            nc.tensor.transpose(pt[:], wt[:, c * P:(c + 1) * P], identr[:])
            dst = w_T8[:, c // 2, c % 2, r * P:(r + 1) * P]
            if c % 2 == 0:
                nc.vector.tensor_copy(dst, pt[:])
            else:
                nc.scalar.activation(dst, pt[:], AF.Copy)


def _emit_A2_phase(nc, T, *, w_T, lhs_blk_fn, a_dram_flat, psum_a, ev_pool):
    """A2 = lhs @ W^T (no bias), lhs supplied per 256-column block by
    lhs_blk_fn (returns SBUF tile [128, KC, 256] = lhs^T block).
    Output rows are (t*BPC + b) flattened, written [T*BPC, G] bf16."""
    n_blocks = (T * BPC) // 256
    for mb in range(n_blocks):
        lhsT_blk = lhs_blk_fn(mb)
        for mt in range(2):
            row0 = mb * 256 + mt * P
            for np_ in range(4):  # pairs of 512-wide n chunks
                pts = [psum_a.tile([P, 512], F32, tag="pa", name=f"pa{j}")
                       for j in range(2)]
                for k in range(KC):
                    for j in range(2):
                        n = np_ * 2 + j
                        nc.tensor.matmul(
                            pts[j][:],
                            lhsT_blk[:, k, mt * P:(mt + 1) * P],
                            w_T[:, k, n * 512:(n + 1) * 512],
                            start=(k == 0), stop=(k == KC - 1),
                        )
                for j in range(2):
                    n = np_ * 2 + j
                    ev = ev_pool.tile([P, 512], ADT, tag="aev")
                    if j == 0:
                        nc.vector.tensor_copy(ev[:], pts[j][:])
                    else:
                        nc.scalar.activation(ev[:], pts[j][:], AF.Copy)
                    nc.sync.dma_start(
                        a_dram_flat[row0:row0 + P, n * 512:(n + 1) * 512], ev[:]
                    )


def _emit_recurrence(nc, T, *, a_dram, bias128, w_T8, hT8, hTb, c_t, a_pool,
                     gact_pool, hpool, psum_g, psum_tr, misc_pool,
                     h1T_dram=None, dec=None, out_ap=None, ident=None,
                     identh=None, a2_fuse=None):
    """T sequential LSTM-cell steps for one layer.

    hT8: fp8 ring [128, KPF, 2, 8, BPC] (k-pair packed, DoubleRow lhsT);
    step t writes slot t%8, reads slot (t-1)%8.  hTb: optional bf16 ring
    [128, KC, 8, BPC] kept in parallel for the h1T flush / A2 path.
    c_t: [BPC, H] fp32 persistent cell state.
    Gate preactivation = PSUM preload (a_t + bias) + h @ W_hh^T (fp8
    DoubleRow: 4 matmuls per 512-wide gate half).
    Weight gate order along G: i, f, g, o.
    """
    a_tiles = {}
    pre_pg = {}

    def load_a(t):
        a1t = a_pool.tile([BPC, G], ADT, tag="a1t", name=f"a1t_{t % 4}")
        nc.sync.dma_start(a1t[:], a_dram[t])
        a_tiles[t] = a1t

    def preload(t, g_idx):
        # preload PSUM with a_t + bias for this gate (vector engine;
        # gpsimd cannot write PSUM)
        pg = psum_g.tile([BPC, H], F32, tag="pg", name=f"pg{g_idx}")
        sl_g = slice(g_idx * H, (g_idx + 1) * H)
        nc.vector.tensor_add(pg[:], a_tiles[t][:, sl_g], bias128[:BPC, sl_g])
        pre_pg[(t, g_idx)] = pg

    def emit_a2_quarter(b, q):
        # quarter q of A2 block b (h1 of steps 4b..4b+3, ring slots
        # (4b)%8..+3): one m-tile x 4 n-chunks = 32 matmuls (~8us), sized
        # to hide in one step's PE tail-idle window
        w_T2, a2_flat, psum_a2, ev2 = a2_fuse
        s0 = (4 * b) % 8
        mt, nh = q // 2, q % 2
        row0 = b * 256 + mt * P
        for n in range(nh * 4, nh * 4 + 4):
            pa = psum_a2.tile([P, 512], F32, tag="pa")
            for k in range(KC):
                nc.tensor.matmul(
                    pa[:], hTb[:, k, s0 + 2 * mt:s0 + 2 * mt + 2, :],
                    w_T2[:, k, n * 512:(n + 1) * 512],
                    start=(k == 0), stop=(k == KC - 1))
            ev = ev2.tile([P, 512], ADT, tag="aev2")
            if n % 2 == 0:
                nc.vector.tensor_copy(ev[:], pa[:])
            else:
                nc.scalar.activation(ev[:], pa[:], AF.Copy)
            nc.sync.dma_start(
                a2_flat[row0:row0 + P, n * 512:(n + 1) * 512], ev[:])

    load_a(0)
    preload(0, 0)
    preload(0, 2)

    for t in range(T):
        if a2_fuse is not None and t >= 4:
            emit_a2_quarter((t - 4) // 4, t % 4)
        s_r = (t + 7) % 8
        s_w = t % 8
        if t + 1 < T:
            load_a(t + 1)
        a1t = a_tiles[t]

        acts = {}

        def mm_gate(g_idx, pg, n2_outer=False):
            loops = ([(n2, kp) for n2 in (1, 0) for kp in range(KPF)] if n2_outer
                     else [(n2, kp) for kp in range(KPF) for n2 in range(2)])
            for n2, kp in loops:
                n = g_idx * 2 + n2
                nc.tensor.matmul(
                    pg[:, n2 * 512:(n2 + 1) * 512],
                    hT8[:, kp, :, s_r, :],
                    w_T8[:, kp, :, n * 512:(n + 1) * 512],
                    start=False, stop=(kp == KPF - 1),
                    perf_mode=mybir.MatmulPerfMode.DoubleRow,
                    skip_group_check=True,
                )

        def do_gate(g_idx, func, tag):
            pg = pre_pg.pop((t, g_idx))
            mm_gate(g_idx, pg)
            at = gact_pool.tile([BPC, H], HDT, tag="gact", name=tag)
            nc.scalar.activation(at[:], pg[:], func)
            acts[g_idx] = at

        HF = 512  # half of H, processed separately to shorten the serial tail
        do_gate(0, AF.Sigmoid, "act_i")        # input gate
        do_gate(2, AF.Tanh, "act_g")           # candidate
        tmp = gact_pool.tile([BPC, H], HDT, tag="gact", name="tmp")
        nc.vector.tensor_mul(tmp[:], acts[0][:], acts[2][:])

        # o/f preloads emitted BEFORE the c-update chain so they don't queue
        # behind it on the vector engine (their banks recycle pg_i / pg_g,
        # whose last readers ran early)
        preload(t, 3)
        preload(t, 1)
        pg_f = pre_pg.pop((t, 1))
        mm_gate(1, pg_f, n2_outer=True)
        act_f = gact_pool.tile([BPC, H], HDT, tag="gact", name="act_f")
        tanh_c = gact_pool.tile([BPC, H], HDT, tag="gact", name="tanh_c")
        for hh in (1, 0):
            # halves split across vector / gpsimd so the two chains run
            # concurrently (all operands SBUF)
            sl = slice(hh * HF, (hh + 1) * HF)
            eng = nc.vector if hh else nc.gpsimd
            nc.scalar.activation(act_f[:, sl], pg_f[:, sl], AF.Sigmoid)
            eng.tensor_mul(c_t[:, sl], c_t[:, sl], act_f[:, sl])
            eng.tensor_add(c_t[:, sl], c_t[:, sl], tmp[:, sl])
            nc.scalar.activation(tanh_c[:, sl], c_t[:, sl], AF.Tanh)

        # output gate + h + h^T, in halves so hT chunks stream out early
        pg_o = pre_pg.pop((t, 3))
        mm_gate(3, pg_o, n2_outer=True)
        act_o = gact_pool.tile([BPC, H], HDT, tag="gact", name="act_o")
        h_t = hpool.tile([BPC, H], HDT, tag="h_t")
        for hh in (1, 0):
            sl = slice(hh * HF, (hh + 1) * HF)
            eng = nc.vector if hh else nc.gpsimd
            nc.scalar.activation(act_o[:, sl], pg_o[:, sl], AF.Sigmoid)
            eng.tensor_mul(h_t[:, sl], act_o[:, sl], tanh_c[:, sl])

        # next step's i/g preloads ride in this step's tail so the PE never
        # waits on a preload at the step boundary
        if t + 1 < T:
            preload(t + 1, 0)
            preload(t + 1, 2)

        # reversed: chunk 0 (needed first by next step) lands last, so the
        # scheduler cannot interleave next-step matmuls with the transposes
        for k in range(KC - 1, -1, -1):
            pt = psum_tr.tile([P, BPC], HDT, tag="htr")
            nc.tensor.transpose(pt[:], h_t[:, k * P:(k + 1) * P], identh[:BPC, :BPC])
            if hTb is not None:
                # bf16 ring for flush/A2; fp8 ring fed from it via gpsimd
                nc.vector.tensor_copy(hTb[:, k, s_w, :], pt[:])
                nc.gpsimd.tensor_copy(hT8[:, k // 2, k % 2, s_w, :],
                                      hTb[:, k, s_w, :])
            else:
                nc.vector.tensor_copy(hT8[:, k // 2, k % 2, s_w, :], pt[:])
                if dec is not None and t == T - 1:
                    # bf16 copy of the final h2^T for a full-precision decode
                    nc.scalar.activation(dec[3][:, k, :], pt[:], AF.Copy)

        if h1T_dram is not None and (s_w == 7 or t == T - 1):
            # flush the ring (contiguous runs per partition)
            nslots = s_w + 1
            col0 = (t // 8) * 8 * BPC
            nc.sync.dma_start(
                h1T_dram.rearrange("(c p) n -> p c n", p=P)[:, :, col0:col0 + nslots * BPC],
                hTb[:, :, 0:nslots, :],
            )

        if dec is not None and t == T - 1:
            decWT, decb_sb, ones_bpc, hT_last = dec
            pd = psum_g.tile([1, BPC], F32, tag="pg", name="pdec")
            for k in range(KC):
                nc.tensor.matmul(pd[:], decWT[:, k:k + 1],
                                 hT_last[:, k, :],
                                 start=(k == 0), stop=False)
            nc.tensor.matmul(pd[:], decb_sb[:], ones_bpc[:],
                             start=False, stop=True)
            osb = misc_pool.tile([1, BPC], F32, tag="osb")
            nc.vector.tensor_copy(osb[:], pd[:])
            nc.sync.dma_start(out_ap.rearrange("b o -> o b"), osb[:])

    if a2_fuse is not None:
        for q in range(4):
            emit_a2_quarter(T // 4 - 1, q)


def build(T=100):
    nc = bacc.Bacc("TRN2", target_bir_lowering=False, debug=False,
                   num_devices=N_CORES)

    x = nc.dram_tensor("x", [BPC, T, IN], F32, kind="ExternalInput").ap()
    enc_W = nc.dram_tensor("enc_W", [H, IN], F32, kind="ExternalInput").ap()
    enc_b = nc.dram_tensor("enc_b", [H], F32, kind="ExternalInput").ap()
    W_ih1 = nc.dram_tensor("W_ih1", [G, H], F32, kind="ExternalInput").ap()
    W_hh1 = nc.dram_tensor("W_hh1", [G, H], F32, kind="ExternalInput").ap()
    b_ih1 = nc.dram_tensor("b_ih1", [G], F32, kind="ExternalInput").ap()
    b_hh1 = nc.dram_tensor("b_hh1", [G], F32, kind="ExternalInput").ap()
    W_ih2 = nc.dram_tensor("W_ih2", [G, H], F32, kind="ExternalInput").ap()
    W_hh2 = nc.dram_tensor("W_hh2", [G, H], F32, kind="ExternalInput").ap()
    b_ih2 = nc.dram_tensor("b_ih2", [G], F32, kind="ExternalInput").ap()
    b_hh2 = nc.dram_tensor("b_hh2", [G], F32, kind="ExternalInput").ap()
    dec_W = nc.dram_tensor("dec_W", [1, H], F32, kind="ExternalInput").ap()
    dec_b = nc.dram_tensor("dec_b", [1], F32, kind="ExternalInput").ap()
    out = nc.dram_tensor("out", [BPC, 1], F32, kind="ExternalOutput").ap()

    A1 = nc.dram_tensor("A1_scratch", [T, BPC, G], ADT).ap()
    A2 = nc.dram_tensor("A2_scratch", [T, BPC, G], ADT).ap()
    h1T = nc.dram_tensor("h1T_scratch", [H, T * BPC], HDT).ap()
    A1_flat = A1.rearrange("t b g -> (t b) g")
    A2_flat = A2.rearrange("t b g -> (t b) g")

    with tile.TileContext(nc) as tc:
        with tc.tile_pool(name="persist", bufs=1) as persist, \
             tc.tile_pool(name="state", bufs=1) as state, \
             tc.tile_pool(name="misc", bufs=1) as misc:

            ident = persist.tile([P, P], F32, tag="ident")
            make_identity(nc, ident[:])
            identr = persist.tile([P, P], F32R, tag="identr")
            nc.vector.tensor_copy(identr[:], ident[:])
            identh = persist.tile([P, P], HDT, tag="identh")
            nc.vector.tensor_copy(identh[:], ident[:])
            ones1 = persist.tile([1, P], F32R, tag="ones1")
            nc.gpsimd.memset(ones1[:].bitcast(F32), 1.0)

            # persistent layer-1 input-side operands
            W_combT = persist.tile([IN, G], F32R, tag="W_combT")
            bias128_1 = persist.tile([P, G], ADT, tag="bias128_1")
            bias128_2 = persist.tile([P, G], ADT, tag="bias128_2")

            def bcast_bias_row(brow, dst, ps_pool):
                """dst[p, n*512:(n+1)*512] = brow[0, n*512:...] for all p."""
                for n in range(8):
                    sl = slice(n * 512, (n + 1) * 512)
                    pb2 = ps_pool.tile([P, 512], F32, tag="pbb")
                    nc.tensor.matmul(pb2[:], ones1[:], brow[:, sl],
                                     start=True, stop=True)
                    nc.vector.tensor_copy(dst[:, sl], pb2[:])

            # ============ Phase E: xT [IN, T*BPC] ============
            with tc.tile_pool(name="xt_pool", bufs=1) as xt_pool:
                xT = xt_pool.tile([IN, T * BPC], F32R, tag="xT")
                with nc.named_scope("phaseE"):
                    with tc.tile_pool(name="e_sb", bufs=3) as e_sb, \
                         tc.tile_pool(name="e_ps", bufs=3, space="PSUM") as e_ps:
                        xr = x.rearrange("b t f -> t b f")
                        for m in range((T * BPC) // P):
                            xt_ = e_sb.tile([P, IN], F32R, tag="xtile")
                            nc.sync.dma_start(xt_[:BPC, :], xr[2 * m].bitcast(F32R))
                            nc.sync.dma_start(xt_[BPC:, :], xr[2 * m + 1].bitcast(F32R))
                            pt = e_ps.tile([IN, P], F32R, tag="xtr")
                            nc.tensor.transpose(pt[:], xt_[:], identr[:])
                            nc.vector.tensor_copy(xT[:, m * P:(m + 1) * P], pt[:])

                # ============ W_combT build (incremental, no 16MB W_ih1^T) ====
                # encwb [128, KC, IN+1]: cols 0..IN-1 = enc_W chunk rows,
                # col IN = enc_b chunk. A single lhsT gives both W_combT rows
                # and the enc_b @ W_ih1^T bias row in one PSUM pass.
                with nc.named_scope("build_Wcomb"):
                    with tc.tile_pool(name="wc_sb", bufs=1) as wc_sb, \
                         tc.tile_pool(name="wc_row", bufs=3) as wc_row, \
                         tc.tile_pool(name="wc_st", bufs=2) as wc_st, \
                         tc.tile_pool(name="wc_ps", bufs=2, space="PSUM") as wc_ps, \
                         tc.tile_pool(name="wc_ps2", bufs=1, space="PSUM") as wc_ps2:
                        encwb = wc_sb.tile([P, KC, IN], F32R, tag="encwb")
                        nc.sync.dma_start(
                            encwb[:],
                            enc_W.rearrange("(c p) f -> p c f", p=P).bitcast(F32R))
                        encb_k = wc_sb.tile([P, KC], F32R, tag="encb_k")
                        nc.sync.dma_start(
                            encb_k[:],
                            enc_b.rearrange("(c p) -> p c", p=P).bitcast(F32R))
                        brow1 = wc_sb.tile([1, G], F32R, tag="brow1")
                        bi1 = wc_sb.tile([1, G], F32, tag="bi1")
                        nc.sync.dma_start(bi1[:], b_ih1[None, :])
                        bh1 = wc_sb.tile([1, G], F32, tag="bh1")
                        nc.sync.dma_start(bh1[:], b_hh1[None, :])
                        # groups of 4 row-chunks = 512 G columns
                        for grp in range(G // 512):
                            wstage = wc_st.tile([P, KC, 512], F32R, tag="wstage")
                            for rr in range(4):
                                r = grp * 4 + rr
                                wt = wc_row.tile([P, H], F32R, tag="wcrow")
                                nc.sync.dma_start(
                                    wt[:], W_ih1[r * P:(r + 1) * P, :].bitcast(F32R))
                                for c in range(KC):
                                    ptr = wc_ps.tile([P, P], F32R, tag="wctr")
                                    nc.tensor.transpose(
                                        ptr[:], wt[:, c * P:(c + 1) * P], identr[:])
                                    if c % 2 == 0:
                                        nc.vector.tensor_copy(
                                            wstage[:, c, rr * P:(rr + 1) * P], ptr[:])
                                    else:
                                        nc.scalar.activation(
                                            wstage[:, c, rr * P:(rr + 1) * P],
                                            ptr[:], AF.Copy)
                            pb = wc_ps2.tile([IN, 512], F32, tag="wcpb")
                            pbias = wc_ps2.tile([1, 512], F32, tag="wcpbias")
                            for k in range(KC):
                                nc.tensor.matmul(pb[:], encwb[:, k, :],
                                                 wstage[:, k, :],
                                                 start=(k == 0), stop=(k == KC - 1))
                            for k in range(KC):
                                nc.tensor.matmul(pbias[:], encb_k[:, k:k + 1],
                                                 wstage[:, k, :],
                                                 start=(k == 0), stop=(k == KC - 1))
                            sl = slice(grp * 512, (grp + 1) * 512)
                            nc.vector.tensor_copy(W_combT[:, sl], pb[:])
                            # bias row = enc_b@W^T + b_ih1 + b_hh1
                            nc.vector.tensor_add(brow1[:, sl], pbias[:],
                                                 bi1[:, sl])
                            nc.gpsimd.tensor_add(brow1[:, sl], brow1[:, sl],
                                                 bh1[:, sl])
                        bcast_bias_row(brow1, bias128_1, wc_ps2)

                # ============ Phase A1: A1 = x @ W_comb^T (K=34) ============
                with nc.named_scope("phaseA1"):
                    with tc.tile_pool(name="a1_ps", bufs=4, space="PSUM") as a_ps, \
                         tc.tile_pool(name="a1_ev", bufs=4) as a_ev:
                        for m in range((T * BPC) // P):
                            for n in range(8):
                                pa = a_ps.tile([P, 512], F32, tag="pa1")
                                nc.tensor.matmul(
                                    pa[:], xT[:, m * P:(m + 1) * P],
                                    W_combT[:, n * 512:(n + 1) * 512],
                                    start=True, stop=True)
                                ev = a_ev.tile([P, 512], ADT, tag="a1ev")
                                if n % 2 == 0:
                                    nc.vector.tensor_copy(ev[:], pa[:])
                                else:
                                    nc.scalar.activation(ev[:], pa[:], AF.Copy)
                                nc.sync.dma_start(
                                    A1_flat[m * P:(m + 1) * P,
                                            n * 512:(n + 1) * 512], ev[:])

            # ===== W_hh1^T + W_ih2^T builds + Phase R1 (A2 fused in) =====
            with tc.tile_pool(name="wpool", bufs=1) as wpool:
                w_T1 = wpool.tile([P, KPF, 2, G], FP8, tag="W8")
                with nc.named_scope("build_Whh1T"):
                    with tc.tile_pool(name="wrow1", bufs=3) as wrow, \
                         tc.tile_pool(name="wtr_ps1", bufs=3, space="PSUM") as wtr_ps:
                        _build_weight_T8(nc, W_hh1, w_T1, identr, wrow, wtr_ps)
                w_T2 = wpool.tile([P, KC, G], WDT, tag="W")
                with nc.named_scope("build_Wih2T"):
                    with tc.tile_pool(name="wrow2", bufs=3) as wrow, \
                         tc.tile_pool(name="wtr_ps2", bufs=3, space="PSUM") as wtr_ps:
                        _build_weight_T(nc, W_ih2, w_T2, identr, wrow, wtr_ps)

                hT8 = state.tile([P, KPF, 2, 8, BPC], FP8, tag="hT8_ring")
                hTb = state.tile([P, KC, 8, BPC], HDT, tag="hTb_ring")
                c_t = state.tile([BPC, H], F32, tag="c_t")
                nc.gpsimd.memset(hT8[:].bitcast(mybir.dt.uint8), 0.0)
                nc.gpsimd.memset(hTb[:].bitcast(mybir.dt.uint16), 0.0)
                nc.gpsimd.memset(c_t[:], 0.0)

                with nc.named_scope("phaseR1"):
                    with tc.tile_pool(name="r1_a", bufs=2) as a_pool, \
                         tc.tile_pool(name="r1_g", bufs=4) as gact_pool, \
                         tc.tile_pool(name="r1_h", bufs=1) as hpool, \
                         tc.tile_pool(name="r1_ev2", bufs=2) as ev2_pool, \
                         tc.tile_pool(name="r1_pg", bufs=2, space="PSUM") as psum_g, \
                         tc.tile_pool(name="r1_ptr", bufs=2, space="PSUM") as psum_tr, \
                         tc.tile_pool(name="r1_pa2", bufs=2, space="PSUM") as psum_a2:
                        _emit_recurrence(nc, T, a_dram=A1, bias128=bias128_1,
                                         w_T8=w_T1, hT8=hT8, hTb=hTb, c_t=c_t,
                                         a_pool=a_pool, gact_pool=gact_pool,
                                         hpool=hpool, psum_g=psum_g,
                                         psum_tr=psum_tr, misc_pool=misc,
                                         h1T_dram=None, ident=ident,
                                         identh=identh,
                                         a2_fuse=(w_T2, A2_flat, psum_a2,
                                                  ev2_pool))

                # layer-2 bias row: b_ih2 + b_hh2 broadcast
                with tc.tile_pool(name="b2_sb", bufs=1) as b2_sb, \
                     tc.tile_pool(name="b2_ps", bufs=2, space="PSUM") as b2_ps:
                    bi2 = b2_sb.tile([1, G], F32, tag="bi2")
                    nc.sync.dma_start(bi2[:], b_ih2[None, :])
                    bh2 = b2_sb.tile([1, G], F32, tag="bh2")
                    nc.sync.dma_start(bh2[:], b_hh2[None, :])
                    brow2 = b2_sb.tile([1, G], F32R, tag="brow2")
                    nc.vector.tensor_add(brow2[:], bi2[:], bh2[:])
                    bcast_bias_row(brow2, bias128_2, b2_ps)

                # ============ W_hh2^T build + Phase R2 (+decode) ============
                w_T3 = wpool.tile([P, KPF, 2, G], FP8, tag="W8")
                with nc.named_scope("build_Whh2T"):
                    with tc.tile_pool(name="wrow3", bufs=3) as wrow, \
                         tc.tile_pool(name="wtr_ps3", bufs=3, space="PSUM") as wtr_ps:
                        _build_weight_T8(nc, W_hh2, w_T3, identr, wrow, wtr_ps)

                nc.gpsimd.memset(hT8[:].bitcast(mybir.dt.uint8), 0.0)
                nc.gpsimd.memset(c_t[:], 0.0)

                decWT_f = misc.tile([P, KC], F32, tag="decWT_f")
                nc.sync.dma_start(decWT_f[:], dec_W.rearrange("o (c p) -> p (c o)", p=P))
                decWT = misc.tile([P, KC], HDT, tag="decWT")
                nc.vector.tensor_copy(decWT[:], decWT_f[:])
                decb_f = misc.tile([1, 1], F32, tag="decb_f")
                nc.sync.dma_start(decb_f[:], dec_b[None, :])
                decb_sb = misc.tile([1, 1], HDT, tag="decb")
                nc.vector.tensor_copy(decb_sb[:], decb_f[:])
                ones_f = misc.tile([1, BPC], F32, tag="ones_f")
                nc.gpsimd.memset(ones_f[:], 1.0)
                ones_bpc = misc.tile([1, BPC], HDT, tag="ones_bpc")
                hT_last = misc.tile([P, KC, BPC], HDT, tag="hT_last")
                nc.vector.tensor_copy(ones_bpc[:], ones_f[:])

                with nc.named_scope("phaseR2"):
                    with tc.tile_pool(name="r2_a", bufs=2) as a_pool, \
                         tc.tile_pool(name="r2_g", bufs=4) as gact_pool, \
                         tc.tile_pool(name="r2_h", bufs=1) as hpool, \
                         tc.tile_pool(name="r2_pg", bufs=3, space="PSUM") as psum_g, \
                         tc.tile_pool(name="r2_ptr", bufs=2, space="PSUM") as psum_tr:
                        _emit_recurrence(nc, T, a_dram=A2, bias128=bias128_2,
                                         w_T8=w_T3, hT8=hT8, hTb=None, c_t=c_t,
                                         a_pool=a_pool, gact_pool=gact_pool,
                                         hpool=hpool, psum_g=psum_g,
                                         psum_tr=psum_tr, misc_pool=misc,
                                         h1T_dram=None,
                                         dec=(decWT, decb_sb, ones_bpc, hT_last),
                                         out_ap=out, ident=ident, identh=identh)

    nc.compile()
    return nc


_cached_nc = None
_cached_fn = None  # (jitted shard_map fn, in_names, out_names, out_shapes, zeros)


def _build_jitted(nc):
    """Same lowering as bass2jax.run_bass_via_pjrt, but the jitted
    executable is cached so repeat kernel() calls skip recompilation."""
    import jax
    from jax.sharding import Mesh, PartitionSpec
    from jax.experimental.shard_map import shard_map
    from concourse import bass2jax, mybir as _mybir

    bass2jax.install_neuronx_cc_hook()
    partition_name = nc.partition_id_tensor.name if nc.partition_id_tensor else None
    in_names, out_names, out_avals, zero_outs = [], [], [], []
    for alloc in nc.m.functions[0].allocations:
        if not isinstance(alloc, _mybir.MemoryLocationSet):
            continue
        name = alloc.memorylocations[0].name
        if alloc.kind == "ExternalInput":
            if name != partition_name:
                in_names.append(name)
        elif alloc.kind == "ExternalOutput":
            shape = tuple(alloc.tensor_shape)
            dtype = _mybir.dt.np(alloc.dtype)
            out_names.append(name)
            out_avals.append(jax.core.ShapedArray(shape, dtype))
            zero_outs.append(np.zeros(shape, dtype))
    n_params = len(in_names)
    n_outs = len(out_avals)
    all_in_names = list(in_names) + list(out_names)
    if partition_name is not None:
        all_in_names.append(partition_name)
    donate = tuple(range(n_params, n_params + n_outs))

    def _body(*args):
        operands = list(args)
        if partition_name is not None:
            operands.append(bass2jax.partition_id_tensor())
        outs = bass2jax._bass_exec_p.bind(
            *operands,
            out_avals=tuple(out_avals),
            in_names=tuple(all_in_names),
            out_names=tuple(out_names),
            lowering_input_output_aliases=(),
            sim_require_finite=True,
            sim_require_nnan=True,
            nc=nc,
        )
        return tuple(outs)

    devices = jax.devices()[:N_CORES]
    mesh = Mesh(np.asarray(devices), ("core",))
    in_specs = (PartitionSpec("core"),) * (n_params + n_outs)
    out_specs = (PartitionSpec("core"),) * n_outs
    fn = jax.jit(
        shard_map(_body, mesh=mesh, in_specs=in_specs, out_specs=out_specs,
                  check_rep=False),
        donate_argnums=donate, keep_unused=True,
    )
    out_shapes = [a.shape for a in out_avals]
    return fn, in_names, out_names, out_shapes, zero_outs


_dev_cache = {}  # name -> (digest, device_array)


def _to_device(name, arr):
    """Replicate-concat a weight to all cores and keep it on device across
    calls (keyed by content hash) so repeat kernel() calls only ship x."""
    import hashlib
    import jax
    d = hashlib.blake2b(arr.tobytes(), digest_size=16).digest()
    hit = _dev_cache.get(name)
    if hit is not None and hit[0] == d:
        return hit[1]
    conc = np.concatenate([arr] * N_CORES, axis=0)
    darr = jax.device_put(conc)
    _dev_cache[name] = (d, darr)
    return darr


def kernel(**inputs):
    global _cached_nc, _cached_fn
    if _cached_nc is None:
        _cached_nc = build(100)
        _cached_fn = _build_jitted(_cached_nc)
    fn, in_names, out_names, out_shapes, zero_outs = _cached_fn
    ins = {k: np.ascontiguousarray(np.asarray(v, dtype=np.float32))
           for k, v in inputs.items()}
    concat_in = []
    for name in in_names:
        if name == "x":
            concat_in.append(ins["x"])  # already [512, T, IN]; axis0 shards
        else:
            concat_in.append(_to_device(name, ins[name]))
    i = out_names.index("out")
    last_err = None
    for attempt in range(3):
        try:
            concat_zeros = [np.zeros((N_CORES * z.shape[0], *z.shape[1:]), z.dtype)
                            for z in zero_outs]
            out_arrs = fn(*concat_in, *concat_zeros)
            outp = np.asarray(out_arrs[i]).reshape(B, 1)
            return outp.astype(np.float32)
        except Exception as e:  # transient NRT_EXEC_UNIT_UNRECOVERABLE etc.
            last_err = e
            _dev_cache.clear()
            concat_in = []
            for name in in_names:
                if name == "x":
                    concat_in.append(ins["x"])
                else:
                    concat_in.append(_to_device(name, ins[name]))
    raise last_err


# revision 31
# speedup vs baseline: 1.2346x; 1.1413x over previous
"""Trainium2 Bass kernel for nn_Discriminator (2-layer LSTM, B=512 T=100 H=1024).

Strategy: data-parallel over batch across 8 cores (B=64 per core).
Per core:
  - Layer-1 input projections collapse through the encoder:
    A1 = x @ (W_ih1 @ enc_W)^T, a [6400,34]@[34,4096] matmul (K=34)
    instead of enc=[6400,1024] then [6400,1024]@[1024,4096].
    W_combT (+ gate bias row) is built incrementally from W_ih1 row
    chunks, so no 16MB transposed copy of W_ih1 is ever materialized.
  - Recurrence weights / h state / A scratch are bf16: half the DMA
    and SBUF traffic and lower PE power (the chip power-throttles the
    PE duty cycle under sustained all-engine load, so energy saved
    anywhere buys PE clock).
  - Gate bias + A-scratch are pre-added into PSUM before the gate
    matmuls (start=False), so activations read PSUM immediately after
    the last matmul.
"""

import numpy as np

import concourse.bass as bass
import concourse.tile as tile
import concourse.mybir as mybir
from concourse import bacc
from concourse.bass_utils import run_bass_kernel_spmd
from concourse.masks import make_identity

F32 = mybir.dt.float32
F32R = mybir.dt.float32r
BF16 = mybir.dt.bfloat16
AF = mybir.ActivationFunctionType

N_CORES = 8
B, IN, H = 512, 34, 1024
G = 4 * H  # 4096
BPC = B // N_CORES  # 64 batch rows per core
P = 128
KC = H // P  # 8 contraction chunks

WDT = BF16   # A2 weight dtype
ADT = BF16   # A1/A2 scratch dtype
HDT = BF16   # h^T ring dtype (flush / A2 path)
FP8 = mybir.dt.float8e4  # recurrence matmul dtype (DoubleRow)
KPF = KC // 2  # fp8 k-pairs per contraction


def _build_weight_T(nc, w_dram, w_T, identr, wrow, wtr_ps):
    """Transpose w_dram [G, H] into resident SBUF tile w_T [128, KC, G]
    (w_T[p, k, j] = W[j, 128k + p]) via PE transposes; w_T may be bf16
    (cast happens in the PSUM->SBUF copy)."""
    n_row_tiles = w_dram.shape[0] // P  # 32
    for r in range(n_row_tiles):
        wt = wrow.tile([P, H], F32R, tag="wrow")
        nc.sync.dma_start(wt[:], w_dram[r * P:(r + 1) * P, :].bitcast(F32R))
        for c in range(KC):
            pt = wtr_ps.tile([P, P], F32R, tag="wtr")
            nc.tensor.transpose(pt[:], wt[:, c * P:(c + 1) * P], identr[:])
            # gpsimd cannot touch PSUM; alternate vector / scalar(Copy)
            if c % 2 == 0:
                nc.vector.tensor_copy(w_T[:, c, r * P:(r + 1) * P], pt[:])
            else:
                nc.scalar.activation(w_T[:, c, r * P:(r + 1) * P], pt[:], AF.Copy)


def _build_weight_T8(nc, w_dram, w_T8, identr, wrow, wtr_ps):
    """Like _build_weight_T but emits fp8 [128, KPF, 2, G] (k-pair packed
    for DoubleRow): w_T8[p, kp, e, j] = W[j, 128*(2*kp+e) + p]."""
    n_row_tiles = w_dram.shape[0] // P  # 32
    for r in range(n_row_tiles):
        wt = wrow.tile([P, H], F32R, tag="wrow")
        nc.sync.dma_start(wt[:], w_dram[r * P:(r + 1) * P, :].bitcast(F32R))
        for c in range(KC):
            pt = wtr_ps.tile([P, P], F32R, tag="wtr")
            nc.tensor.transpose(pt[:], wt[:, c * P:(c + 1) * P], identr[:])
            dst = w_T8[:, c // 2, c % 2, r * P:(r + 1) * P]
            if c % 2 == 0:
                nc.vector.tensor_copy(dst, pt[:])
            else:
                nc.scalar.activation(dst, pt[:], AF.Copy)


def _emit_A2_phase(nc, T, *, w_T, bias128, lhs_blk_fn, a_dram_flat, psum_a, ev_pool):
    """A2 = lhs @ W^T (no bias), lhs supplied per 256-column block by
    lhs_blk_fn (returns SBUF tile [128, KC, 256] = lhs^T block).
    Output rows are (t*BPC + b) flattened, written [T*BPC, G] bf16."""
    n_blocks = (T * BPC) // 256
    for mb in range(n_blocks):
        lhsT_blk = lhs_blk_fn(mb)
        for mt in range(2):
            row0 = mb * 256 + mt * P
            for np_ in range(4):  # pairs of 512-wide n chunks
                pts = [psum_a.tile([P, 512], F32, tag="pa", name=f"pa{j}")
                       for j in range(2)]
                for k in range(KC):
                    for j in range(2):
                        n = np_ * 2 + j
                        nc.tensor.matmul(
                            pts[j][:],
                            lhsT_blk[:, k, mt * P:(mt + 1) * P],
                            w_T[:, k, n * 512:(n + 1) * 512],
                            start=(k == 0), stop=(k == KC - 1),
                        )
                for j in range(2):
                    n = np_ * 2 + j
                    ev = ev_pool.tile([P, 512], ADT, tag="aev")
                    nc.vector.tensor_add(ev[:], pts[j][:],
                                         bias128[:, n * 512:(n + 1) * 512])
                    nc.sync.dma_start(
                        a_dram_flat[row0:row0 + P, n * 512:(n + 1) * 512], ev[:]
                    )


def _emit_recurrence(nc, T, *, a_dram, w_T8, hT8, hTb, c_t, a_pool,
                     gact_pool, hpool, psum_g, psum_tr, misc_pool,
                     h1T_dram=None, dec=None, out_ap=None, ident=None,
                     identh=None):
    """T sequential LSTM-cell steps for one layer.

    hT8: fp8 ring [128, KPF, 2, 8, BPC] (k-pair packed, DoubleRow lhsT);
    step t writes slot t%8, reads slot (t-1)%8.  hTb: optional bf16 ring
    [128, KC, 8, BPC] kept in parallel for the h1T flush / A2 path.
    c_t: [BPC, H] fp32 persistent cell state.
    Gate preactivation = PSUM preload (a_t + bias) + h @ W_hh^T (fp8
    DoubleRow: 4 matmuls per 512-wide gate half).
    Weight gate order along G: i, f, g, o.
    """
    a_tiles = {}

    def load_a(t):
        a1t = a_pool.tile([BPC, G], ADT, tag="a1t", name=f"a1t_{t % 4}")
        nc.sync.dma_start(a1t[:], a_dram[t])
        a_tiles[t] = a1t

    load_a(0)

    for t in range(T):
        s_r = (t + 7) % 8
        s_w = t % 8
        if t + 1 < T:
            load_a(t + 1)
        a1t = a_tiles[t]

        acts = {}

        def mm_gate(g_idx, pg, n2_outer=False):
            # a_t (bias already folded in) enters PSUM via an identity
            # matmul on the PE itself: no cross-engine preload sync, and
            # the vector engine stays free for the c/h chains
            for n2 in range(2):
                n = g_idx * 2 + n2
                nc.tensor.matmul(
                    pg[:, n2 * 512:(n2 + 1) * 512],
                    identh[:BPC, :BPC],
                    a1t[:, n * 512:(n + 1) * 512],
                    start=True, stop=False, skip_group_check=True,
                )
            loops = ([(n2, kp) for n2 in (1, 0) for kp in range(KPF)] if n2_outer
                     else [(n2, kp) for kp in range(KPF) for n2 in range(2)])
            for n2, kp in loops:
                n = g_idx * 2 + n2
                nc.tensor.matmul(
                    pg[:, n2 * 512:(n2 + 1) * 512],
                    hT8[:, kp, :, s_r, :],
                    w_T8[:, kp, :, n * 512:(n + 1) * 512],
                    start=False, stop=(kp == KPF - 1),
                    perf_mode=mybir.MatmulPerfMode.DoubleRow,
                    skip_group_check=True,
                )

        def do_gate(g_idx, func, tag):
            pg = psum_g.tile([BPC, H], F32, tag="pg", name=f"pg{g_idx}")
            mm_gate(g_idx, pg)
            at = gact_pool.tile([BPC, H], HDT, tag="gact", name=tag)
            nc.scalar.activation(at[:], pg[:], func)
            acts[g_idx] = at

        HF = 512  # half of H, processed separately to shorten the serial tail
        do_gate(0, AF.Sigmoid, "act_i")        # input gate
        do_gate(2, AF.Tanh, "act_g")           # candidate
        tmp = gact_pool.tile([BPC, H], HDT, tag="gact", name="tmp")
        nc.vector.tensor_mul(tmp[:], acts[0][:], acts[2][:])

        pg_f = psum_g.tile([BPC, H], F32, tag="pg", name="pg_f")
        mm_gate(1, pg_f, n2_outer=True)
        act_f = gact_pool.tile([BPC, H], HDT, tag="gact", name="act_f")
        tanh_c = gact_pool.tile([BPC, H], HDT, tag="gact", name="tanh_c")
        for hh in (1, 0):
            sl = slice(hh * HF, (hh + 1) * HF)
            nc.scalar.activation(act_f[:, sl], pg_f[:, sl], AF.Sigmoid)
            nc.vector.tensor_mul(c_t[:, sl], c_t[:, sl], act_f[:, sl])
            nc.vector.tensor_add(c_t[:, sl], c_t[:, sl], tmp[:, sl])
            nc.scalar.activation(tanh_c[:, sl], c_t[:, sl], AF.Tanh)

        # keepalive: the PE drops to the 1.2GHz p-state during the tail idle
        # and takes ~10 matmuls to recover; a tiny matmul keeps it hot
        ka1 = psum_tr.tile([1, 256], F32, tag="htr", name="ka1")
        nc.tensor.matmul(ka1[:], identh[:BPC, 0:1], tanh_c[:, 0:256],
                         start=True, stop=True)

        # output gate + h + h^T, in halves so hT chunks stream out early
        pg_o = psum_g.tile([BPC, H], F32, tag="pg", name="pg_o")
        mm_gate(3, pg_o, n2_outer=True)
        act_o = gact_pool.tile([BPC, H], HDT, tag="gact", name="act_o")
        h_t = hpool.tile([BPC, H], HDT, tag="h_t")
        for hh in (1, 0):
            sl = slice(hh * HF, (hh + 1) * HF)
            nc.scalar.activation(act_o[:, sl], pg_o[:, sl], AF.Sigmoid)
            nc.vector.tensor_mul(h_t[:, sl], act_o[:, sl], tanh_c[:, sl])
        ka2 = psum_tr.tile([1, 256], F32, tag="htr", name="ka2")
        nc.tensor.matmul(ka2[:], identh[:BPC, 0:1], tanh_c[:, 256:512],
                         start=True, stop=True)

        # reversed: chunk 0 (needed first by next step) lands last, so the
        # scheduler cannot interleave next-step matmuls with the transposes
        for k in range(KC - 1, -1, -1):
            pt = psum_tr.tile([P, BPC], HDT, tag="htr")
            nc.tensor.transpose(pt[:], h_t[:, k * P:(k + 1) * P], identh[:BPC, :BPC])
            if hTb is not None:
                # bf16 ring for flush/A2; fp8 ring fed from it via gpsimd
                nc.vector.tensor_copy(hTb[:, k, s_w, :], pt[:])
                nc.gpsimd.tensor_copy(hT8[:, k // 2, k % 2, s_w, :],
                                      hTb[:, k, s_w, :])
            else:
                nc.vector.tensor_copy(hT8[:, k // 2, k % 2, s_w, :], pt[:])
                if dec is not None and t == T - 1:
                    # bf16 copy of the final h2^T for a full-precision decode
                    nc.scalar.activation(dec[3][:, k, :], pt[:], AF.Copy)

        if h1T_dram is not None and (s_w == 7 or t == T - 1):
            # flush the ring (contiguous runs per partition)
            nslots = s_w + 1
            col0 = (t // 8) * 8 * BPC
            nc.sync.dma_start(
                h1T_dram.rearrange("(c p) n -> p c n", p=P)[:, :, col0:col0 + nslots * BPC],
                hTb[:, :, 0:nslots, :],
            )

        if dec is not None and t == T - 1:
            decWT, decb_sb, ones_bpc, hT_last = dec
            pd = psum_g.tile([1, BPC], F32, tag="pg", name="pdec")
            for k in range(KC):
                nc.tensor.matmul(pd[:], decWT[:, k:k + 1],
                                 hT_last[:, k, :],
                                 start=(k == 0), stop=False)
            nc.tensor.matmul(pd[:], decb_sb[:], ones_bpc[:],
                             start=False, stop=True)
            osb = misc_pool.tile([1, BPC], F32, tag="osb")
            nc.vector.tensor_copy(osb[:], pd[:])
            nc.sync.dma_start(out_ap.rearrange("b o -> o b"), osb[:])


def build(T=100):
    nc = bacc.Bacc("TRN2", target_bir_lowering=False, debug=False,
                   num_devices=N_CORES)

    x = nc.dram_tensor("x", [BPC, T, IN], F32, kind="ExternalInput").ap()
    enc_W = nc.dram_tensor("enc_W", [H, IN], F32, kind="ExternalInput").ap()
    enc_b = nc.dram_tensor("enc_b", [H], F32, kind="ExternalInput").ap()
    W_ih1 = nc.dram_tensor("W_ih1", [G, H], F32, kind="ExternalInput").ap()
    W_hh1 = nc.dram_tensor("W_hh1", [G, H], F32, kind="ExternalInput").ap()
    b_ih1 = nc.dram_tensor("b_ih1", [G], F32, kind="ExternalInput").ap()
    b_hh1 = nc.dram_tensor("b_hh1", [G], F32, kind="ExternalInput").ap()
    W_ih2 = nc.dram_tensor("W_ih2", [G, H], F32, kind="ExternalInput").ap()
    W_hh2 = nc.dram_tensor("W_hh2", [G, H], F32, kind="ExternalInput").ap()
    b_ih2 = nc.dram_tensor("b_ih2", [G], F32, kind="ExternalInput").ap()
    b_hh2 = nc.dram_tensor("b_hh2", [G], F32, kind="ExternalInput").ap()
    dec_W = nc.dram_tensor("dec_W", [1, H], F32, kind="ExternalInput").ap()
    dec_b = nc.dram_tensor("dec_b", [1], F32, kind="ExternalInput").ap()
    out = nc.dram_tensor("out", [BPC, 1], F32, kind="ExternalOutput").ap()

    A1 = nc.dram_tensor("A1_scratch", [T, BPC, G], ADT).ap()
    A2 = nc.dram_tensor("A2_scratch", [T, BPC, G], ADT).ap()
    h1T = nc.dram_tensor("h1T_scratch", [H, T * BPC], HDT).ap()
    A1_flat = A1.rearrange("t b g -> (t b) g")
    A2_flat = A2.rearrange("t b g -> (t b) g")

    with tile.TileContext(nc) as tc:
        with tc.tile_pool(name="persist", bufs=1) as persist, \
             tc.tile_pool(name="state", bufs=1) as state, \
             tc.tile_pool(name="misc", bufs=1) as misc:

            ident = persist.tile([P, P], F32, tag="ident")
            make_identity(nc, ident[:])
            identr = persist.tile([P, P], F32R, tag="identr")
            nc.vector.tensor_copy(identr[:], ident[:])
            identh = persist.tile([P, P], HDT, tag="identh")
            nc.vector.tensor_copy(identh[:], ident[:])
            ones1 = persist.tile([1, P], F32R, tag="ones1")
            nc.gpsimd.memset(ones1[:].bitcast(F32), 1.0)

            # persistent layer-1 input-side operands
            W_combT = persist.tile([IN, G], F32R, tag="W_combT")
            bias128_1 = persist.tile([P, G], ADT, tag="bias128_1")
            bias128_2 = persist.tile([P, G], ADT, tag="bias128_2")

            def bcast_bias_row(brow, dst, ps_pool):
                """dst[p, n*512:(n+1)*512] = brow[0, n*512:...] for all p."""
                for n in range(8):
                    sl = slice(n * 512, (n + 1) * 512)
                    pb2 = ps_pool.tile([P, 512], F32, tag="pbb")
                    nc.tensor.matmul(pb2[:], ones1[:], brow[:, sl],
                                     start=True, stop=True)
                    nc.vector.tensor_copy(dst[:, sl], pb2[:])

            # ============ Phase E: xT [IN, T*BPC] ============
            with tc.tile_pool(name="xt_pool", bufs=1) as xt_pool:
                xT = xt_pool.tile([IN, T * BPC], F32R, tag="xT")
                with nc.named_scope("phaseE"):
                    with tc.tile_pool(name="e_sb", bufs=3) as e_sb, \
                         tc.tile_pool(name="e_ps", bufs=3, space="PSUM") as e_ps:
                        xr = x.rearrange("b t f -> t b f")
                        for m in range((T * BPC) // P):
                            xt_ = e_sb.tile([P, IN], F32R, tag="xtile")
                            nc.sync.dma_start(xt_[:BPC, :], xr[2 * m].bitcast(F32R))
                            nc.sync.dma_start(xt_[BPC:, :], xr[2 * m + 1].bitcast(F32R))
                            pt = e_ps.tile([IN, P], F32R, tag="xtr")
                            nc.tensor.transpose(pt[:], xt_[:], identr[:])
                            nc.vector.tensor_copy(xT[:, m * P:(m + 1) * P], pt[:])

                # ============ W_combT build (incremental, no 16MB W_ih1^T) ====
                # encwb [128, KC, IN+1]: cols 0..IN-1 = enc_W chunk rows,
                # col IN = enc_b chunk. A single lhsT gives both W_combT rows
                # and the enc_b @ W_ih1^T bias row in one PSUM pass.
                with nc.named_scope("build_Wcomb"):
                    with tc.tile_pool(name="wc_sb", bufs=1) as wc_sb, \
                         tc.tile_pool(name="wc_row", bufs=3) as wc_row, \
                         tc.tile_pool(name="wc_st", bufs=2) as wc_st, \
                         tc.tile_pool(name="wc_ps", bufs=2, space="PSUM") as wc_ps, \
                         tc.tile_pool(name="wc_ps2", bufs=1, space="PSUM") as wc_ps2:
                        encwb = wc_sb.tile([P, KC, IN], F32R, tag="encwb")
                        nc.sync.dma_start(
                            encwb[:],
                            enc_W.rearrange("(c p) f -> p c f", p=P).bitcast(F32R))
                        encb_k = wc_sb.tile([P, KC], F32R, tag="encb_k")
                        nc.sync.dma_start(
                            encb_k[:],
                            enc_b.rearrange("(c p) -> p c", p=P).bitcast(F32R))
                        brow1 = wc_sb.tile([1, G], F32R, tag="brow1")
                        bi1 = wc_sb.tile([1, G], F32, tag="bi1")
                        nc.sync.dma_start(bi1[:], b_ih1[None, :])
                        bh1 = wc_sb.tile([1, G], F32, tag="bh1")
                        nc.sync.dma_start(bh1[:], b_hh1[None, :])
                        # groups of 4 row-chunks = 512 G columns
                        for grp in range(G // 512):
                            wstage = wc_st.tile([P, KC, 512], F32R, tag="wstage")
                            for rr in range(4):
                                r = grp * 4 + rr
                                wt = wc_row.tile([P, H], F32R, tag="wcrow")
                                nc.sync.dma_start(
                                    wt[:], W_ih1[r * P:(r + 1) * P, :].bitcast(F32R))
                                for c in range(KC):
                                    ptr = wc_ps.tile([P, P], F32R, tag="wctr")
                                    nc.tensor.transpose(
                                        ptr[:], wt[:, c * P:(c + 1) * P], identr[:])
                                    if c % 2 == 0:
                                        nc.vector.tensor_copy(
                                            wstage[:, c, rr * P:(rr + 1) * P], ptr[:])
                                    else:
                                        nc.scalar.activation(
                                            wstage[:, c, rr * P:(rr + 1) * P],
                                            ptr[:], AF.Copy)
                            pb = wc_ps2.tile([IN, 512], F32, tag="wcpb")
                            pbias = wc_ps2.tile([1, 512], F32, tag="wcpbias")
                            for k in range(KC):
                                nc.tensor.matmul(pb[:], encwb[:, k, :],
                                                 wstage[:, k, :],
                                                 start=(k == 0), stop=(k == KC - 1))
                            for k in range(KC):
                                nc.tensor.matmul(pbias[:], encb_k[:, k:k + 1],
                                                 wstage[:, k, :],
                                                 start=(k == 0), stop=(k == KC - 1))
                            sl = slice(grp * 512, (grp + 1) * 512)
                            nc.vector.tensor_copy(W_combT[:, sl], pb[:])
                            # bias row = enc_b@W^T + b_ih1 + b_hh1
                            nc.vector.tensor_add(brow1[:, sl], pbias[:],
                                                 bi1[:, sl])
                            nc.gpsimd.tensor_add(brow1[:, sl], brow1[:, sl],
                                                 bh1[:, sl])
                        bcast_bias_row(brow1, bias128_1, wc_ps2)

                # ============ Phase A1: A1 = x @ W_comb^T (K=34) ============
                with nc.named_scope("phaseA1"):
                    with tc.tile_pool(name="a1_ps", bufs=4, space="PSUM") as a_ps, \
                         tc.tile_pool(name="a1_ev", bufs=4) as a_ev:
                        for m in range((T * BPC) // P):
                            for n in range(8):
                                pa = a_ps.tile([P, 512], F32, tag="pa1")
                                nc.tensor.matmul(
                                    pa[:], xT[:, m * P:(m + 1) * P],
                                    W_combT[:, n * 512:(n + 1) * 512],
                                    start=True, stop=True)
                                ev = a_ev.tile([P, 512], ADT, tag="a1ev")
                                nc.vector.tensor_add(
                                    ev[:], pa[:],
                                    bias128_1[:, n * 512:(n + 1) * 512])
                                nc.sync.dma_start(
                                    A1_flat[m * P:(m + 1) * P,
                                            n * 512:(n + 1) * 512], ev[:])

            # ============ W_hh1^T build + Phase R1 ============
            with tc.tile_pool(name="wpool", bufs=1) as wpool:
                w_T1 = wpool.tile([P, KPF, 2, G], FP8, tag="W8")
                with nc.named_scope("build_Whh1T"):
                    with tc.tile_pool(name="wrow1", bufs=3) as wrow, \
                         tc.tile_pool(name="wtr_ps1", bufs=3, space="PSUM") as wtr_ps:
                        _build_weight_T8(nc, W_hh1, w_T1, identr, wrow, wtr_ps)

                hT8 = state.tile([P, KPF, 2, 8, BPC], FP8, tag="hT8_ring")
                hTb = state.tile([P, KC, 8, BPC], HDT, tag="hTb_ring")
                c_t = state.tile([BPC, H], F32, tag="c_t")
                nc.gpsimd.memset(hT8[:].bitcast(mybir.dt.uint8), 0.0)
                nc.gpsimd.memset(hTb[:].bitcast(mybir.dt.uint16), 0.0)
                nc.gpsimd.memset(c_t[:], 0.0)

                with nc.named_scope("phaseR1"):
                    with tc.tile_pool(name="r1_a", bufs=2) as a_pool, \
                         tc.tile_pool(name="r1_g", bufs=4) as gact_pool, \
                         tc.tile_pool(name="r1_h", bufs=1) as hpool, \
                         tc.tile_pool(name="r1_pg", bufs=3, space="PSUM") as psum_g, \
                         tc.tile_pool(name="r1_ptr", bufs=2, space="PSUM") as psum_tr:
                        _emit_recurrence(nc, T, a_dram=A1,
                                         w_T8=w_T1, hT8=hT8, hTb=hTb, c_t=c_t,
                                         a_pool=a_pool, gact_pool=gact_pool,
                                         hpool=hpool, psum_g=psum_g,
                                         psum_tr=psum_tr, misc_pool=misc,
                                         h1T_dram=h1T, ident=ident,
                                         identh=identh)

                # ============ W_ih2^T build + Phase A2 ============
                w_T2 = wpool.tile([P, KC, G], WDT, tag="W")
                with nc.named_scope("build_Wih2T"):
                    with tc.tile_pool(name="wrow2", bufs=3) as wrow, \
                         tc.tile_pool(name="wtr_ps2", bufs=3, space="PSUM") as wtr_ps:
                        _build_weight_T(nc, W_ih2, w_T2, identr, wrow, wtr_ps)

                with nc.named_scope("phaseA2"):
                    # layer-2 bias row: b_ih2 + b_hh2 broadcast
                    with tc.tile_pool(name="b2_sb", bufs=1) as b2_sb, \
                         tc.tile_pool(name="b2_ps", bufs=2, space="PSUM") as b2_ps:
                        bi2 = b2_sb.tile([1, G], F32, tag="bi2")
                        nc.sync.dma_start(bi2[:], b_ih2[None, :])
                        bh2 = b2_sb.tile([1, G], F32, tag="bh2")
                        nc.sync.dma_start(bh2[:], b_hh2[None, :])
                        brow2 = b2_sb.tile([1, G], F32R, tag="brow2")
                        nc.vector.tensor_add(brow2[:], bi2[:], bh2[:])
                        bcast_bias_row(brow2, bias128_2, b2_ps)

                    h1T_r = h1T.rearrange("(c p) n -> p c n", p=P)
                    with tc.tile_pool(name="h1_sb", bufs=2) as h1_sb, \
                         tc.tile_pool(name="a_ps2", bufs=4, space="PSUM") as a_ps, \
                         tc.tile_pool(name="a_ev2", bufs=2) as a_ev:

                        def h1_blk(mb):
                            blk = h1_sb.tile([P, KC, 256], HDT, tag="h1blk")
                            nc.sync.dma_start(
                                blk[:], h1T_r[:, :, mb * 256:(mb + 1) * 256])
                            return blk

                        _emit_A2_phase(nc, T, w_T=w_T2, bias128=bias128_2,
                                       lhs_blk_fn=h1_blk,
                                       a_dram_flat=A2_flat,
                                       psum_a=a_ps, ev_pool=a_ev)

                # ============ W_hh2^T build + Phase R2 (+decode) ============
                w_T3 = wpool.tile([P, KPF, 2, G], FP8, tag="W8")
                with nc.named_scope("build_Whh2T"):
                    with tc.tile_pool(name="wrow3", bufs=3) as wrow, \
                         tc.tile_pool(name="wtr_ps3", bufs=3, space="PSUM") as wtr_ps:
                        _build_weight_T8(nc, W_hh2, w_T3, identr, wrow, wtr_ps)

                nc.gpsimd.memset(hT8[:].bitcast(mybir.dt.uint8), 0.0)
                nc.gpsimd.memset(c_t[:], 0.0)

                decWT_f = misc.tile([P, KC], F32, tag="decWT_f")
                nc.sync.dma_start(decWT_f[:], dec_W.rearrange("o (c p) -> p (c o)", p=P))
                decWT = misc.tile([P, KC], HDT, tag="decWT")
                nc.vector.tensor_copy(decWT[:], decWT_f[:])
                decb_f = misc.tile([1, 1], F32, tag="decb_f")
                nc.sync.dma_start(decb_f[:], dec_b[None, :])
                decb_sb = misc.tile([1, 1], HDT, tag="decb")
                nc.vector.tensor_copy(decb_sb[:], decb_f[:])
                ones_f = misc.tile([1, BPC], F32, tag="ones_f")
                nc.gpsimd.memset(ones_f[:], 1.0)
                ones_bpc = misc.tile([1, BPC], HDT, tag="ones_bpc")
                hT_last = misc.tile([P, KC, BPC], HDT, tag="hT_last")
                nc.vector.tensor_copy(ones_bpc[:], ones_f[:])

                with nc.named_scope("phaseR2"):
                    with tc.tile_pool(name="r2_a", bufs=2) as a_pool, \
                         tc.tile_pool(name="r2_g", bufs=4) as gact_pool, \
                         tc.tile_pool(name="r2_h", bufs=1) as hpool, \
                         tc.tile_pool(name="r2_pg", bufs=3, space="PSUM") as psum_g, \
                         tc.tile_pool(name="r2_ptr", bufs=2, space="PSUM") as psum_tr:
                        _emit_recurrence(nc, T, a_dram=A2,
                                         w_T8=w_T3, hT8=hT8, hTb=None, c_t=c_t,
                                         a_pool=a_pool, gact_pool=gact_pool,
                                         hpool=hpool, psum_g=psum_g,
                                         psum_tr=psum_tr, misc_pool=misc,
                                         h1T_dram=None,
                                         dec=(decWT, decb_sb, ones_bpc, hT_last),
                                         out_ap=out, ident=ident, identh=identh)

    nc.compile()
    return nc


_cached_nc = None
_cached_fn = None  # (jitted shard_map fn, in_names, out_names, out_shapes, zeros)


def _build_jitted(nc):
    """Same lowering as bass2jax.run_bass_via_pjrt, but the jitted
    executable is cached so repeat kernel() calls skip recompilation."""
    import jax
    from jax.sharding import Mesh, PartitionSpec
    from jax.experimental.shard_map import shard_map
    from concourse import bass2jax, mybir as _mybir

    bass2jax.install_neuronx_cc_hook()
    partition_name = nc.partition_id_tensor.name if nc.partition_id_tensor else None
    in_names, out_names, out_avals, zero_outs = [], [], [], []
    for alloc in nc.m.functions[0].allocations:
        if not isinstance(alloc, _mybir.MemoryLocationSet):
            continue
        name = alloc.memorylocations[0].name
        if alloc.kind == "ExternalInput":
            if name != partition_name:
                in_names.append(name)
        elif alloc.kind == "ExternalOutput":
            shape = tuple(alloc.tensor_shape)
            dtype = _mybir.dt.np(alloc.dtype)
            out_names.append(name)
            out_avals.append(jax.core.ShapedArray(shape, dtype))
            zero_outs.append(np.zeros(shape, dtype))
    n_params = len(in_names)
    n_outs = len(out_avals)
    all_in_names = list(in_names) + list(out_names)
    if partition_name is not None:
        all_in_names.append(partition_name)
    donate = tuple(range(n_params, n_params + n_outs))

    def _body(*args):
        operands = list(args)
        if partition_name is not None:
            operands.append(bass2jax.partition_id_tensor())
        outs = bass2jax._bass_exec_p.bind(
            *operands,
            out_avals=tuple(out_avals),
            in_names=tuple(all_in_names),
            out_names=tuple(out_names),
            lowering_input_output_aliases=(),
            sim_require_finite=True,
            sim_require_nnan=True,
            nc=nc,
        )
        return tuple(outs)

    devices = jax.devices()[:N_CORES]
    mesh = Mesh(np.asarray(devices), ("core",))
    in_specs = (PartitionSpec("core"),) * (n_params + n_outs)
    out_specs = (PartitionSpec("core"),) * n_outs
    fn = jax.jit(
        shard_map(_body, mesh=mesh, in_specs=in_specs, out_specs=out_specs,
                  check_rep=False),
        donate_argnums=donate, keep_unused=True,
    )
    out_shapes = [a.shape for a in out_avals]
    return fn, in_names, out_names, out_shapes, zero_outs


_dev_cache = {}  # name -> (digest, device_array)


def _to_device(name, arr):
    """Replicate-concat a weight to all cores and keep it on device across
    calls (keyed by content hash) so repeat kernel() calls only ship x."""
    import hashlib
    import jax
    d = hashlib.blake2b(arr.tobytes(), digest_size=16).digest()
    hit = _dev_cache.get(name)
    if hit is not None and hit[0] == d:
        return hit[1]
    conc = np.concatenate([arr] * N_CORES, axis=0)
    darr = jax.device_put(conc)
    _dev_cache[name] = (d, darr)
    return darr


def kernel(**inputs):
    global _cached_nc, _cached_fn
    if _cached_nc is None:
        _cached_nc = build(100)
        _cached_fn = _build_jitted(_cached_nc)
    fn, in_names, out_names, out_shapes, zero_outs = _cached_fn
    ins = {k: np.ascontiguousarray(np.asarray(v, dtype=np.float32))
           for k, v in inputs.items()}
    concat_in = []
    for name in in_names:
        if name == "x":
            concat_in.append(ins["x"])  # already [512, T, IN]; axis0 shards
        else:
            concat_in.append(_to_device(name, ins[name]))
    i = out_names.index("out")
    last_err = None
    for attempt in range(3):
        try:
            concat_zeros = [np.zeros((N_CORES * z.shape[0], *z.shape[1:]), z.dtype)
                            for z in zero_outs]
            out_arrs = fn(*concat_in, *concat_zeros)
            outp = np.asarray(out_arrs[i]).reshape(B, 1)
            return outp.astype(np.float32)
        except Exception as e:  # transient NRT_EXEC_UNIT_UNRECOVERABLE etc.
            last_err = e
            _dev_cache.clear()
            concat_in = []
            for name in in_names:
                if name == "x":
                    concat_in.append(ins["x"])
                else:
                    concat_in.append(_to_device(name, ins[name]))
    raise last_err


# revision 32
# speedup vs baseline: 1.4766x; 1.1961x over previous
"""Trainium2 Bass kernel for nn_Discriminator (2-layer LSTM, B=512 T=100 H=1024).

Strategy: data-parallel over batch across 8 cores (B=64 per core).
Per core:
  - Layer-1 input projections collapse through the encoder:
    A1 = x @ (W_ih1 @ enc_W)^T, a [6400,34]@[34,4096] matmul (K=34)
    instead of enc=[6400,1024] then [6400,1024]@[1024,4096].
    W_combT (+ gate bias row) is built incrementally from W_ih1 row
    chunks, so no 16MB transposed copy of W_ih1 is ever materialized.
  - Recurrence weights / h state / A scratch are bf16: half the DMA
    and SBUF traffic and lower PE power (the chip power-throttles the
    PE duty cycle under sustained all-engine load, so energy saved
    anywhere buys PE clock).
  - Gate bias + A-scratch are pre-added into PSUM before the gate
    matmuls (start=False), so activations read PSUM immediately after
    the last matmul.
"""

import numpy as np

import concourse.bass as bass
import concourse.tile as tile
import concourse.mybir as mybir
from concourse import bacc
from concourse.bass_utils import run_bass_kernel_spmd
from concourse.masks import make_identity

F32 = mybir.dt.float32
F32R = mybir.dt.float32r
BF16 = mybir.dt.bfloat16
AF = mybir.ActivationFunctionType

N_CORES = 8
B, IN, H = 512, 34, 1024
G = 4 * H  # 4096
BPC = B // N_CORES  # 64 batch rows per core
P = 128
KC = H // P  # 8 contraction chunks

WDT = BF16   # A2 weight dtype
ADT = BF16   # A1/A2 scratch dtype
HDT = BF16   # h^T ring dtype (flush / A2 path)
FP8 = mybir.dt.float8e4  # recurrence matmul dtype (DoubleRow)
KPF = KC // 2  # fp8 k-pairs per contraction


def _build_weight_T(nc, w_dram, w_T, identr, wrow, wtr_ps):
    """Transpose w_dram [G, H] into resident SBUF tile w_T [128, KC, G]
    (w_T[p, k, j] = W[j, 128k + p]) via PE transposes; w_T may be bf16
    (cast happens in the PSUM->SBUF copy)."""
    n_row_tiles = w_dram.shape[0] // P  # 32
    for r in range(n_row_tiles):
        wt = wrow.tile([P, H], F32R, tag="wrow")
        nc.sync.dma_start(wt[:], w_dram[r * P:(r + 1) * P, :].bitcast(F32R))
        for c in range(KC):
            pt = wtr_ps.tile([P, P], F32R, tag="wtr")
            nc.tensor.transpose(pt[:], wt[:, c * P:(c + 1) * P], identr[:])
            # gpsimd cannot touch PSUM; alternate vector / scalar(Copy)
            if c % 2 == 0:
                nc.vector.tensor_copy(w_T[:, c, r * P:(r + 1) * P], pt[:])
            else:
                nc.scalar.activation(w_T[:, c, r * P:(r + 1) * P], pt[:], AF.Copy)


def _build_weight_T8(nc, w_dram, w_T8, identr, wrow, wtr_ps):
    """Like _build_weight_T but emits fp8 [128, KPF, 2, G] (k-pair packed
    for DoubleRow): w_T8[p, kp, e, j] = W[j, 128*(2*kp+e) + p]."""
    n_row_tiles = w_dram.shape[0] // P  # 32
    for r in range(n_row_tiles):
        wt = wrow.tile([P, H], F32R, tag="wrow")
        nc.sync.dma_start(wt[:], w_dram[r * P:(r + 1) * P, :].bitcast(F32R))
        for c in range(KC):
            pt = wtr_ps.tile([P, P], F32R, tag="wtr")
            nc.tensor.transpose(pt[:], wt[:, c * P:(c + 1) * P], identr[:])
            dst = w_T8[:, c // 2, c % 2, r * P:(r + 1) * P]
            if c % 2 == 0:
                nc.vector.tensor_copy(dst, pt[:])
            else:
                nc.scalar.activation(dst, pt[:], AF.Copy)


def _emit_A2_phase(nc, T, *, w_T, bias128, lhs_blk_fn, a_dram_flat, psum_a, ev_pool):
    """A2 = lhs @ W^T (no bias), lhs supplied per 256-column block by
    lhs_blk_fn (returns SBUF tile [128, KC, 256] = lhs^T block).
    Output rows are (t*BPC + b) flattened, written [T*BPC, G] bf16."""
    n_blocks = (T * BPC) // 256
    for mb in range(n_blocks):
        lhsT_blk = lhs_blk_fn(mb)
        for mt in range(2):
            row0 = mb * 256 + mt * P
            for np_ in range(4):  # pairs of 512-wide n chunks
                pts = [psum_a.tile([P, 512], F32, tag="pa", name=f"pa{j}")
                       for j in range(2)]
                for k in range(KC):
                    for j in range(2):
                        n = np_ * 2 + j
                        nc.tensor.matmul(
                            pts[j][:],
                            lhsT_blk[:, k, mt * P:(mt + 1) * P],
                            w_T[:, k, n * 512:(n + 1) * 512],
                            start=(k == 0), stop=(k == KC - 1),
                        )
                for j in range(2):
                    n = np_ * 2 + j
                    ev = ev_pool.tile([P, 512], ADT, tag="aev")
                    nc.vector.tensor_add(ev[:], pts[j][:],
                                         bias128[:, n * 512:(n + 1) * 512])
                    nc.sync.dma_start(
                        a_dram_flat[row0:row0 + P, n * 512:(n + 1) * 512], ev[:]
                    )


def _emit_recurrence(nc, T, *, a_dram, w_T8, hT8, hTb, c_t, a_pool,
                     gact_pool, hpool, psum_g, psum_tr, misc_pool,
                     h1T_dram=None, dec=None, out_ap=None, ident=None,
                     identh=None):
    """T sequential LSTM-cell steps for one layer.

    hT8: fp8 ring [128, KPF, 2, 8, BPC] (k-pair packed, DoubleRow lhsT);
    step t writes slot t%8, reads slot (t-1)%8.  hTb: optional bf16 ring
    [128, KC, 8, BPC] kept in parallel for the h1T flush / A2 path.
    c_t: [BPC, H] fp32 persistent cell state.
    Gate preactivation = PSUM preload (a_t + bias) + h @ W_hh^T (fp8
    DoubleRow: 4 matmuls per 512-wide gate half).
    Weight gate order along G: i, f, g, o.
    """
    a_tiles = {}

    def load_a(t):
        a1t = a_pool.tile([BPC, G], ADT, tag="a1t", name=f"a1t_{t % 4}")
        nc.sync.dma_start(a1t[:], a_dram[t])
        a_tiles[t] = a1t

    load_a(0)
    pg_next = {}

    for t in range(T):
        s_r = (t + 7) % 8
        s_w = t % 8
        if t + 1 < T:
            load_a(t + 1)
        a1t = a_tiles[t]

        acts = {}

        def preload_pe(g_idx, pg, at):
            # a_t (bias already folded in) enters PSUM via an identity
            # matmul on the PE itself: no cross-engine preload sync, and
            # the vector engine stays free for the c/h chains
            for n2 in range(2):
                n = g_idx * 2 + n2
                nc.tensor.matmul(
                    pg[:, n2 * 512:(n2 + 1) * 512],
                    identh[:BPC, :BPC],
                    at[:, n * 512:(n + 1) * 512],
                    start=True, stop=False, skip_group_check=True,
                )

        def mm_gate(g_idx, pg, n2_outer=False, preloaded=False):
            if not preloaded:
                preload_pe(g_idx, pg, a1t)
            loops = ([(n2, kp) for n2 in (1, 0) for kp in range(KPF)] if n2_outer
                     else [(n2, kp) for kp in range(KPF) for n2 in range(2)])
            for n2, kp in loops:
                n = g_idx * 2 + n2
                nc.tensor.matmul(
                    pg[:, n2 * 512:(n2 + 1) * 512],
                    hT8[:, kp, :, s_r, :],
                    w_T8[:, kp, :, n * 512:(n + 1) * 512],
                    start=False, stop=(kp == KPF - 1),
                    perf_mode=mybir.MatmulPerfMode.DoubleRow,
                    skip_group_check=True,
                )

        def do_gate(g_idx, func, tag):
            pg = pg_next.pop(g_idx, None)
            if pg is None:
                pg = psum_g.tile([BPC, H], F32, tag="pg", name=f"pg{g_idx}")
                mm_gate(g_idx, pg)
            else:
                mm_gate(g_idx, pg, preloaded=True)
            at = gact_pool.tile([BPC, H], HDT, tag="gact", name=tag)
            nc.scalar.activation(at[:], pg[:], func)
            acts[g_idx] = at

        HF = 512  # half of H, processed separately to shorten the serial tail
        do_gate(0, AF.Sigmoid, "act_i")        # input gate
        do_gate(2, AF.Tanh, "act_g")           # candidate
        tmp = gact_pool.tile([BPC, H], HDT, tag="gact", name="tmp")
        nc.vector.tensor_mul(tmp[:], acts[0][:], acts[2][:])

        pg_f = psum_g.tile([BPC, H], F32, tag="pg", name="pg_f")
        mm_gate(1, pg_f, n2_outer=True)
        act_f = gact_pool.tile([BPC, H], HDT, tag="gact", name="act_f")
        tanh_c = gact_pool.tile([BPC, H], HDT, tag="gact", name="tanh_c")
        for hh in (1, 0):
            sl = slice(hh * HF, (hh + 1) * HF)
            nc.scalar.activation(act_f[:, sl], pg_f[:, sl], AF.Sigmoid)
            nc.vector.tensor_mul(c_t[:, sl], c_t[:, sl], act_f[:, sl])
            nc.vector.tensor_add(c_t[:, sl], c_t[:, sl], tmp[:, sl])
            nc.scalar.activation(tanh_c[:, sl], c_t[:, sl], AF.Tanh)

        # keepalive: the PE drops to the 1.2GHz p-state during the tail idle
        # and takes ~10 matmuls to recover; a tiny matmul keeps it hot
        ka1 = psum_tr.tile([1, 256], F32, tag="htr", name="ka1")
        nc.tensor.matmul(ka1[:], identh[:BPC, 0:1], tanh_c[:, 0:256],
                         start=True, stop=True)

        # output gate + h + h^T, in halves so hT chunks stream out early
        pg_o = psum_g.tile([BPC, H], F32, tag="pg", name="pg_o")
        mm_gate(3, pg_o, n2_outer=True)
        act_o = gact_pool.tile([BPC, H], HDT, tag="gact", name="act_o")
        h_t = hpool.tile([BPC, H], HDT, tag="h_t")
        for hh in (1, 0):
            sl = slice(hh * HF, (hh + 1) * HF)
            nc.scalar.activation(act_o[:, sl], pg_o[:, sl], AF.Sigmoid)
            nc.vector.tensor_mul(h_t[:, sl], act_o[:, sl], tanh_c[:, sl])
        ka2 = psum_tr.tile([1, 256], F32, tag="htr", name="ka2")
        nc.tensor.matmul(ka2[:], identh[:BPC, 0:1], tanh_c[:, 256:512],
                         start=True, stop=True)

        # gate-i preload of step t+1 rides before the transposes: the PE
        # does it while waiting on h_t, and the post-transpose restart goes
        # straight into DoubleRow matmuls
        if t + 1 < T:
            pg_i = psum_g.tile([BPC, H], F32, tag="pg", name="pg0")
            preload_pe(0, pg_i, a_tiles[t + 1])
            pg_next[0] = pg_i

        # reversed: chunk 0 (needed first by next step) lands last, so the
        # scheduler cannot interleave next-step matmuls with the transposes
        for k in range(KC - 1, -1, -1):
            pt = psum_tr.tile([P, BPC], HDT, tag="htr")
            nc.tensor.transpose(pt[:], h_t[:, k * P:(k + 1) * P], identh[:BPC, :BPC])
            if hTb is not None:
                # critical fp8 ring via vector; bf16 flush/A2 ring via the
                # scalar engine (both read the transpose PSUM in parallel)
                nc.vector.tensor_copy(hT8[:, k // 2, k % 2, s_w, :], pt[:])
                nc.scalar.activation(hTb[:, k, s_w, :], pt[:], AF.Copy)
            else:
                nc.vector.tensor_copy(hT8[:, k // 2, k % 2, s_w, :], pt[:])
                if dec is not None and t == T - 1:
                    # bf16 copy of the final h2^T for a full-precision decode
                    nc.scalar.activation(dec[3][:, k, :], pt[:], AF.Copy)

        if h1T_dram is not None and (s_w == 7 or t == T - 1):
            # flush the ring (contiguous runs per partition)
            nslots = s_w + 1
            col0 = (t // 8) * 8 * BPC
            nc.sync.dma_start(
                h1T_dram.rearrange("(c p) n -> p c n", p=P)[:, :, col0:col0 + nslots * BPC],
                hTb[:, :, 0:nslots, :],
            )

        if dec is not None and t == T - 1:
            decWT, decb_sb, ones_bpc, hT_last = dec
            pd = psum_g.tile([1, BPC], F32, tag="pg", name="pdec")
            for k in range(KC):
                nc.tensor.matmul(pd[:], decWT[:, k:k + 1],
                                 hT_last[:, k, :],
                                 start=(k == 0), stop=False)
            nc.tensor.matmul(pd[:], decb_sb[:], ones_bpc[:],
                             start=False, stop=True)
            osb = misc_pool.tile([1, BPC], F32, tag="osb")
            nc.vector.tensor_copy(osb[:], pd[:])
            nc.sync.dma_start(out_ap.rearrange("b o -> o b"), osb[:])


def build(T=100):
    nc = bacc.Bacc("TRN2", target_bir_lowering=False, debug=False,
                   num_devices=N_CORES)

    x = nc.dram_tensor("x", [BPC, T, IN], F32, kind="ExternalInput").ap()
    enc_W = nc.dram_tensor("enc_W", [H, IN], F32, kind="ExternalInput").ap()
    enc_b = nc.dram_tensor("enc_b", [H], F32, kind="ExternalInput").ap()
    W_ih1 = nc.dram_tensor("W_ih1", [G, H], F32, kind="ExternalInput").ap()
    W_hh1 = nc.dram_tensor("W_hh1", [G, H], F32, kind="ExternalInput").ap()
    b_ih1 = nc.dram_tensor("b_ih1", [G], F32, kind="ExternalInput").ap()
    b_hh1 = nc.dram_tensor("b_hh1", [G], F32, kind="ExternalInput").ap()
    W_ih2 = nc.dram_tensor("W_ih2", [G, H], F32, kind="ExternalInput").ap()
    W_hh2 = nc.dram_tensor("W_hh2", [G, H], F32, kind="ExternalInput").ap()
    b_ih2 = nc.dram_tensor("b_ih2", [G], F32, kind="ExternalInput").ap()
    b_hh2 = nc.dram_tensor("b_hh2", [G], F32, kind="ExternalInput").ap()
    dec_W = nc.dram_tensor("dec_W", [1, H], F32, kind="ExternalInput").ap()
    dec_b = nc.dram_tensor("dec_b", [1], F32, kind="ExternalInput").ap()
    out = nc.dram_tensor("out", [BPC, 1], F32, kind="ExternalOutput").ap()

    A1 = nc.dram_tensor("A1_scratch", [T, BPC, G], ADT).ap()
    A2 = nc.dram_tensor("A2_scratch", [T, BPC, G], ADT).ap()
    h1T = nc.dram_tensor("h1T_scratch", [H, T * BPC], HDT).ap()
    A1_flat = A1.rearrange("t b g -> (t b) g")
    A2_flat = A2.rearrange("t b g -> (t b) g")

    with tile.TileContext(nc) as tc:
        with tc.tile_pool(name="persist", bufs=1) as persist, \
             tc.tile_pool(name="state", bufs=1) as state, \
             tc.tile_pool(name="misc", bufs=1) as misc:

            ident = persist.tile([P, P], F32, tag="ident")
            make_identity(nc, ident[:])
            identr = persist.tile([P, P], F32R, tag="identr")
            nc.vector.tensor_copy(identr[:], ident[:])
            identh = persist.tile([P, P], HDT, tag="identh")
            nc.vector.tensor_copy(identh[:], ident[:])
            ones1 = persist.tile([1, P], F32R, tag="ones1")
            nc.gpsimd.memset(ones1[:].bitcast(F32), 1.0)

            # persistent layer-1 input-side operands
            W_combT = persist.tile([IN, G], F32R, tag="W_combT")
            bias128_1 = persist.tile([P, G], ADT, tag="bias128_1")
            bias128_2 = persist.tile([P, G], ADT, tag="bias128_2")

            def bcast_bias_row(brow, dst, ps_pool):
                """dst[p, n*512:(n+1)*512] = brow[0, n*512:...] for all p."""
                for n in range(8):
                    sl = slice(n * 512, (n + 1) * 512)
                    pb2 = ps_pool.tile([P, 512], F32, tag="pbb")
                    nc.tensor.matmul(pb2[:], ones1[:], brow[:, sl],
                                     start=True, stop=True)
                    nc.vector.tensor_copy(dst[:, sl], pb2[:])

            # ============ Phase E: xT [IN, T*BPC] ============
            with tc.tile_pool(name="xt_pool", bufs=1) as xt_pool:
                xT = xt_pool.tile([IN, T * BPC], F32R, tag="xT")
                with nc.named_scope("phaseE"):
                    with tc.tile_pool(name="e_sb", bufs=3) as e_sb, \
                         tc.tile_pool(name="e_ps", bufs=3, space="PSUM") as e_ps:
                        xr = x.rearrange("b t f -> t b f")
                        for m in range((T * BPC) // P):
                            xt_ = e_sb.tile([P, IN], F32R, tag="xtile")
                            nc.sync.dma_start(xt_[:BPC, :], xr[2 * m].bitcast(F32R))
                            nc.sync.dma_start(xt_[BPC:, :], xr[2 * m + 1].bitcast(F32R))
                            pt = e_ps.tile([IN, P], F32R, tag="xtr")
                            nc.tensor.transpose(pt[:], xt_[:], identr[:])
                            nc.vector.tensor_copy(xT[:, m * P:(m + 1) * P], pt[:])

                # ============ W_combT build (incremental, no 16MB W_ih1^T) ====
                # encwb [128, KC, IN+1]: cols 0..IN-1 = enc_W chunk rows,
                # col IN = enc_b chunk. A single lhsT gives both W_combT rows
                # and the enc_b @ W_ih1^T bias row in one PSUM pass.
                with nc.named_scope("build_Wcomb"):
                    with tc.tile_pool(name="wc_sb", bufs=1) as wc_sb, \
                         tc.tile_pool(name="wc_row", bufs=3) as wc_row, \
                         tc.tile_pool(name="wc_st", bufs=2) as wc_st, \
                         tc.tile_pool(name="wc_ps", bufs=2, space="PSUM") as wc_ps, \
                         tc.tile_pool(name="wc_ps2", bufs=1, space="PSUM") as wc_ps2:
                        encwb = wc_sb.tile([P, KC, IN], F32R, tag="encwb")
                        nc.sync.dma_start(
                            encwb[:],
                            enc_W.rearrange("(c p) f -> p c f", p=P).bitcast(F32R))
                        encb_k = wc_sb.tile([P, KC], F32R, tag="encb_k")
                        nc.sync.dma_start(
                            encb_k[:],
                            enc_b.rearrange("(c p) -> p c", p=P).bitcast(F32R))
                        brow1 = wc_sb.tile([1, G], F32R, tag="brow1")
                        bi1 = wc_sb.tile([1, G], F32, tag="bi1")
                        nc.sync.dma_start(bi1[:], b_ih1[None, :])
                        bh1 = wc_sb.tile([1, G], F32, tag="bh1")
                        nc.sync.dma_start(bh1[:], b_hh1[None, :])
                        # groups of 4 row-chunks = 512 G columns
                        for grp in range(G // 512):
                            wstage = wc_st.tile([P, KC, 512], F32R, tag="wstage")
                            for rr in range(4):
                                r = grp * 4 + rr
                                wt = wc_row.tile([P, H], F32R, tag="wcrow")
                                nc.sync.dma_start(
                                    wt[:], W_ih1[r * P:(r + 1) * P, :].bitcast(F32R))
                                for c in range(KC):
                                    ptr = wc_ps.tile([P, P], F32R, tag="wctr")
                                    nc.tensor.transpose(
                                        ptr[:], wt[:, c * P:(c + 1) * P], identr[:])
                                    if c % 2 == 0:
                                        nc.vector.tensor_copy(
                                            wstage[:, c, rr * P:(rr + 1) * P], ptr[:])
                                    else:
                                        nc.scalar.activation(
                                            wstage[:, c, rr * P:(rr + 1) * P],
                                            ptr[:], AF.Copy)
                            pb = wc_ps2.tile([IN, 512], F32, tag="wcpb")
                            pbias = wc_ps2.tile([1, 512], F32, tag="wcpbias")
                            for k in range(KC):
                                nc.tensor.matmul(pb[:], encwb[:, k, :],
                                                 wstage[:, k, :],
                                                 start=(k == 0), stop=(k == KC - 1))
                            for k in range(KC):
                                nc.tensor.matmul(pbias[:], encb_k[:, k:k + 1],
                                                 wstage[:, k, :],
                                                 start=(k == 0), stop=(k == KC - 1))
                            sl = slice(grp * 512, (grp + 1) * 512)
                            nc.vector.tensor_copy(W_combT[:, sl], pb[:])
                            # bias row = enc_b@W^T + b_ih1 + b_hh1
                            nc.vector.tensor_add(brow1[:, sl], pbias[:],
                                                 bi1[:, sl])
                            nc.gpsimd.tensor_add(brow1[:, sl], brow1[:, sl],
                                                 bh1[:, sl])
                        bcast_bias_row(brow1, bias128_1, wc_ps2)

                # ============ Phase A1: A1 = x @ W_comb^T (K=34) ============
                with nc.named_scope("phaseA1"):
                    with tc.tile_pool(name="a1_ps", bufs=4, space="PSUM") as a_ps, \
                         tc.tile_pool(name="a1_ev", bufs=4) as a_ev:
                        for m in range((T * BPC) // P):
                            for n in range(8):
                                pa = a_ps.tile([P, 512], F32, tag="pa1")
                                nc.tensor.matmul(
                                    pa[:], xT[:, m * P:(m + 1) * P],
                                    W_combT[:, n * 512:(n + 1) * 512],
                                    start=True, stop=True)
                                ev = a_ev.tile([P, 512], ADT, tag="a1ev")
                                nc.vector.tensor_add(
                                    ev[:], pa[:],
                                    bias128_1[:, n * 512:(n + 1) * 512])
                                nc.sync.dma_start(
                                    A1_flat[m * P:(m + 1) * P,
                                            n * 512:(n + 1) * 512], ev[:])

            # ============ W_hh1^T build + Phase R1 ============
            with tc.tile_pool(name="wpool", bufs=1) as wpool:
                w_T1 = wpool.tile([P, KPF, 2, G], FP8, tag="W8")
                with nc.named_scope("build_Whh1T"):
                    with tc.tile_pool(name="wrow1", bufs=3) as wrow, \
                         tc.tile_pool(name="wtr_ps1", bufs=3, space="PSUM") as wtr_ps:
                        _build_weight_T8(nc, W_hh1, w_T1, identr, wrow, wtr_ps)

                hT8 = state.tile([P, KPF, 2, 8, BPC], FP8, tag="hT8_ring")
                hTb = state.tile([P, KC, 8, BPC], HDT, tag="hTb_ring")
                c_t = state.tile([BPC, H], F32, tag="c_t")
                nc.gpsimd.memset(hT8[:].bitcast(mybir.dt.uint8), 0.0)
                nc.gpsimd.memset(hTb[:].bitcast(mybir.dt.uint16), 0.0)
                nc.gpsimd.memset(c_t[:], 0.0)

                with nc.named_scope("phaseR1"):
                    with tc.tile_pool(name="r1_a", bufs=2) as a_pool, \
                         tc.tile_pool(name="r1_g", bufs=4) as gact_pool, \
                         tc.tile_pool(name="r1_h", bufs=1) as hpool, \
                         tc.tile_pool(name="r1_pg", bufs=3, space="PSUM") as psum_g, \
                         tc.tile_pool(name="r1_ptr", bufs=2, space="PSUM") as psum_tr:
                        _emit_recurrence(nc, T, a_dram=A1,
                                         w_T8=w_T1, hT8=hT8, hTb=hTb, c_t=c_t,
                                         a_pool=a_pool, gact_pool=gact_pool,
                                         hpool=hpool, psum_g=psum_g,
                                         psum_tr=psum_tr, misc_pool=misc,
                                         h1T_dram=h1T, ident=ident,
                                         identh=identh)

                # ============ W_ih2^T build + Phase A2 ============
                w_T2 = wpool.tile([P, KC, G], WDT, tag="W")
                with nc.named_scope("build_Wih2T"):
                    with tc.tile_pool(name="wrow2", bufs=3) as wrow, \
                         tc.tile_pool(name="wtr_ps2", bufs=3, space="PSUM") as wtr_ps:
                        _build_weight_T(nc, W_ih2, w_T2, identr, wrow, wtr_ps)

                with nc.named_scope("phaseA2"):
                    # layer-2 bias row: b_ih2 + b_hh2 broadcast
                    with tc.tile_pool(name="b2_sb", bufs=1) as b2_sb, \
                         tc.tile_pool(name="b2_ps", bufs=2, space="PSUM") as b2_ps:
                        bi2 = b2_sb.tile([1, G], F32, tag="bi2")
                        nc.sync.dma_start(bi2[:], b_ih2[None, :])
                        bh2 = b2_sb.tile([1, G], F32, tag="bh2")
                        nc.sync.dma_start(bh2[:], b_hh2[None, :])
                        brow2 = b2_sb.tile([1, G], F32R, tag="brow2")
                        nc.vector.tensor_add(brow2[:], bi2[:], bh2[:])
                        bcast_bias_row(brow2, bias128_2, b2_ps)

                    h1T_r = h1T.rearrange("(c p) n -> p c n", p=P)
                    with tc.tile_pool(name="h1_sb", bufs=2) as h1_sb, \
                         tc.tile_pool(name="a_ps2", bufs=4, space="PSUM") as a_ps, \
                         tc.tile_pool(name="a_ev2", bufs=2) as a_ev:

                        def h1_blk(mb):
                            blk = h1_sb.tile([P, KC, 256], HDT, tag="h1blk")
                            nc.sync.dma_start(
                                blk[:], h1T_r[:, :, mb * 256:(mb + 1) * 256])
                            return blk

                        _emit_A2_phase(nc, T, w_T=w_T2, bias128=bias128_2,
                                       lhs_blk_fn=h1_blk,
                                       a_dram_flat=A2_flat,
                                       psum_a=a_ps, ev_pool=a_ev)

                # ============ W_hh2^T build + Phase R2 (+decode) ============
                w_T3 = wpool.tile([P, KPF, 2, G], FP8, tag="W8")
                with nc.named_scope("build_Whh2T"):
                    with tc.tile_pool(name="wrow3", bufs=3) as wrow, \
                         tc.tile_pool(name="wtr_ps3", bufs=3, space="PSUM") as wtr_ps:
                        _build_weight_T8(nc, W_hh2, w_T3, identr, wrow, wtr_ps)

                nc.gpsimd.memset(hT8[:].bitcast(mybir.dt.uint8), 0.0)
                nc.gpsimd.memset(c_t[:], 0.0)

                decWT_f = misc.tile([P, KC], F32, tag="decWT_f")
                nc.sync.dma_start(decWT_f[:], dec_W.rearrange("o (c p) -> p (c o)", p=P))
                decWT = misc.tile([P, KC], HDT, tag="decWT")
                nc.vector.tensor_copy(decWT[:], decWT_f[:])
                decb_f = misc.tile([1, 1], F32, tag="decb_f")
                nc.sync.dma_start(decb_f[:], dec_b[None, :])
                decb_sb = misc.tile([1, 1], HDT, tag="decb")
                nc.vector.tensor_copy(decb_sb[:], decb_f[:])
                ones_f = misc.tile([1, BPC], F32, tag="ones_f")
                nc.gpsimd.memset(ones_f[:], 1.0)
                ones_bpc = misc.tile([1, BPC], HDT, tag="ones_bpc")
                hT_last = misc.tile([P, KC, BPC], HDT, tag="hT_last")
                nc.vector.tensor_copy(ones_bpc[:], ones_f[:])

                with nc.named_scope("phaseR2"):
                    with tc.tile_pool(name="r2_a", bufs=2) as a_pool, \
                         tc.tile_pool(name="r2_g", bufs=4) as gact_pool, \
                         tc.tile_pool(name="r2_h", bufs=1) as hpool, \
                         tc.tile_pool(name="r2_pg", bufs=3, space="PSUM") as psum_g, \
                         tc.tile_pool(name="r2_ptr", bufs=2, space="PSUM") as psum_tr:
                        _emit_recurrence(nc, T, a_dram=A2,
                                         w_T8=w_T3, hT8=hT8, hTb=None, c_t=c_t,
                                         a_pool=a_pool, gact_pool=gact_pool,
                                         hpool=hpool, psum_g=psum_g,
                                         psum_tr=psum_tr, misc_pool=misc,
                                         h1T_dram=None,
                                         dec=(decWT, decb_sb, ones_bpc, hT_last),
                                         out_ap=out, ident=ident, identh=identh)

    nc.compile()
    return nc


_cached_nc = None
_cached_fn = None  # (jitted shard_map fn, in_names, out_names, out_shapes, zeros)


def _build_jitted(nc):
    """Same lowering as bass2jax.run_bass_via_pjrt, but the jitted
    executable is cached so repeat kernel() calls skip recompilation."""
    import jax
    from jax.sharding import Mesh, PartitionSpec
    from jax.experimental.shard_map import shard_map
    from concourse import bass2jax, mybir as _mybir

    bass2jax.install_neuronx_cc_hook()
    partition_name = nc.partition_id_tensor.name if nc.partition_id_tensor else None
    in_names, out_names, out_avals, zero_outs = [], [], [], []
    for alloc in nc.m.functions[0].allocations:
        if not isinstance(alloc, _mybir.MemoryLocationSet):
            continue
        name = alloc.memorylocations[0].name
        if alloc.kind == "ExternalInput":
            if name != partition_name:
                in_names.append(name)
        elif alloc.kind == "ExternalOutput":
            shape = tuple(alloc.tensor_shape)
            dtype = _mybir.dt.np(alloc.dtype)
            out_names.append(name)
            out_avals.append(jax.core.ShapedArray(shape, dtype))
            zero_outs.append(np.zeros(shape, dtype))
    n_params = len(in_names)
    n_outs = len(out_avals)
    all_in_names = list(in_names) + list(out_names)
    if partition_name is not None:
        all_in_names.append(partition_name)
    donate = tuple(range(n_params, n_params + n_outs))

    def _body(*args):
        operands = list(args)
        if partition_name is not None:
            operands.append(bass2jax.partition_id_tensor())
        outs = bass2jax._bass_exec_p.bind(
            *operands,
            out_avals=tuple(out_avals),
            in_names=tuple(all_in_names),
            out_names=tuple(out_names),
            lowering_input_output_aliases=(),
            sim_require_finite=True,
            sim_require_nnan=True,
            nc=nc,
        )
        return tuple(outs)

    devices = jax.devices()[:N_CORES]
    mesh = Mesh(np.asarray(devices), ("core",))
    in_specs = (PartitionSpec("core"),) * (n_params + n_outs)
    out_specs = (PartitionSpec("core"),) * n_outs
    fn = jax.jit(
        shard_map(_body, mesh=mesh, in_specs=in_specs, out_specs=out_specs,
                  check_rep=False),
        donate_argnums=donate, keep_unused=True,
    )
    out_shapes = [a.shape for a in out_avals]
    return fn, in_names, out_names, out_shapes, zero_outs


_dev_cache = {}  # name -> (digest, device_array)


def _to_device(name, arr):
    """Replicate-concat a weight to all cores and keep it on device across
    calls (keyed by content hash) so repeat kernel() calls only ship x."""
    import hashlib
    import jax
    d = hashlib.blake2b(arr.tobytes(), digest_size=16).digest()
    hit = _dev_cache.get(name)
    if hit is not None and hit[0] == d:
        return hit[1]
    conc = np.concatenate([arr] * N_CORES, axis=0)
    darr = jax.device_put(conc)
    _dev_cache[name] = (d, darr)
    return darr


def kernel(**inputs):
    global _cached_nc, _cached_fn
    if _cached_nc is None:
        _cached_nc = build(100)
        _cached_fn = _build_jitted(_cached_nc)
    fn, in_names, out_names, out_shapes, zero_outs = _cached_fn
    ins = {k: np.ascontiguousarray(np.asarray(v, dtype=np.float32))
           for k, v in inputs.items()}
    concat_in = []
    for name in in_names:
        if name == "x":
            concat_in.append(ins["x"])  # already [512, T, IN]; axis0 shards
        else:
            concat_in.append(_to_device(name, ins[name]))
    i = out_names.index("out")
    last_err = None
    for attempt in range(3):
        try:
            concat_zeros = [np.zeros((N_CORES * z.shape[0], *z.shape[1:]), z.dtype)
                            for z in zero_outs]
            out_arrs = fn(*concat_in, *concat_zeros)
            outp = np.asarray(out_arrs[i]).reshape(B, 1)
            return outp.astype(np.float32)
        except Exception as e:  # transient NRT_EXEC_UNIT_UNRECOVERABLE etc.
            last_err = e
            _dev_cache.clear()
            concat_in = []
            for name in in_names:
                if name == "x":
                    concat_in.append(ins["x"])
                else:
                    concat_in.append(_to_device(name, ins[name]))
    raise last_err


# revision 35
# speedup vs baseline: 1.5859x; 1.0740x over previous
"""Trainium2 Bass kernel for nn_Discriminator (2-layer LSTM, B=512 T=100 H=1024).

Strategy: data-parallel over batch across 8 cores (B=64 per core).
Per core:
  - Layer-1 input projections collapse through the encoder:
    A1 = x @ (W_ih1 @ enc_W)^T, a [6400,34]@[34,4096] matmul (K=34)
    instead of enc=[6400,1024] then [6400,1024]@[1024,4096].
    W_combT (+ gate bias row) is built incrementally from W_ih1 row
    chunks, so no 16MB transposed copy of W_ih1 is ever materialized.
  - Recurrence weights / h state / A scratch are bf16: half the DMA
    and SBUF traffic and lower PE power (the chip power-throttles the
    PE duty cycle under sustained all-engine load, so energy saved
    anywhere buys PE clock).
  - Gate bias + A-scratch are pre-added into PSUM before the gate
    matmuls (start=False), so activations read PSUM immediately after
    the last matmul.
"""

import numpy as np

import concourse.bass as bass
import concourse.tile as tile
import concourse.mybir as mybir
from concourse import bacc
from concourse.bass_utils import run_bass_kernel_spmd
from concourse.masks import make_identity

F32 = mybir.dt.float32
F32R = mybir.dt.float32r
BF16 = mybir.dt.bfloat16
AF = mybir.ActivationFunctionType

N_CORES = 8
B, IN, H = 512, 34, 1024
G = 4 * H  # 4096
BPC = B // N_CORES  # 64 batch rows per core
P = 128
KC = H // P  # 8 contraction chunks

WDT = BF16   # A2 weight dtype
ADT = BF16   # A1/A2 scratch dtype
HDT = BF16   # h^T ring dtype (flush / A2 path)
FP8 = mybir.dt.float8e4  # recurrence matmul dtype (DoubleRow)
KPF = KC // 2  # fp8 k-pairs per contraction


def _build_weight_T(nc, w_dram, w_T, identr, wrow, wtr_ps):
    """Transpose w_dram [G, H] into resident SBUF tile w_T [128, KC, G]
    (w_T[p, k, j] = W[j, 128k + p]) via PE transposes; w_T may be bf16
    (cast happens in the PSUM->SBUF copy)."""
    n_row_tiles = w_dram.shape[0] // P  # 32
    for r in range(n_row_tiles):
        wt = wrow.tile([P, H], F32R, tag="wrow")
        nc.sync.dma_start(wt[:], w_dram[r * P:(r + 1) * P, :].bitcast(F32R))
        for c in range(KC):
            pt = wtr_ps.tile([P, P], F32R, tag="wtr")
            nc.tensor.transpose(pt[:], wt[:, c * P:(c + 1) * P], identr[:])
            # gpsimd cannot touch PSUM; alternate vector / scalar(Copy)
            if c % 2 == 0:
                nc.vector.tensor_copy(w_T[:, c, r * P:(r + 1) * P], pt[:])
            else:
                nc.scalar.activation(w_T[:, c, r * P:(r + 1) * P], pt[:], AF.Copy)


def _build_weight_T8(nc, w_dram, w_T8, identr, wrow, wtr_ps):
    """Like _build_weight_T but emits fp8 [128, KPF, 2, G] (k-pair packed
    for DoubleRow): w_T8[p, kp, e, j] = W[j, 128*(2*kp+e) + p]."""
    n_row_tiles = w_dram.shape[0] // P  # 32
    for r in range(n_row_tiles):
        wt = wrow.tile([P, H], F32R, tag="wrow")
        nc.sync.dma_start(wt[:], w_dram[r * P:(r + 1) * P, :].bitcast(F32R))
        for c in range(KC):
            pt = wtr_ps.tile([P, P], F32R, tag="wtr")
            nc.tensor.transpose(pt[:], wt[:, c * P:(c + 1) * P], identr[:])
            dst = w_T8[:, c // 2, c % 2, r * P:(r + 1) * P]
            if c % 2 == 0:
                nc.vector.tensor_copy(dst, pt[:])
            else:
                nc.scalar.activation(dst, pt[:], AF.Copy)


def _emit_A2_phase(nc, T, *, w_T, bias128, lhs_blk_fn, a_dram_flat, psum_a, ev_pool):
    """A2 = lhs @ W^T (no bias), lhs supplied per 256-column block by
    lhs_blk_fn (returns SBUF tile [128, KC, 256] = lhs^T block).
    Output rows are (t*BPC + b) flattened, written [T*BPC, G] bf16."""
    n_blocks = (T * BPC) // 256
    for mb in range(n_blocks):
        lhsT_blk = lhs_blk_fn(mb)
        for mt in range(2):
            row0 = mb * 256 + mt * P
            for np_ in range(4):  # pairs of 512-wide n chunks
                pts = [psum_a.tile([P, 512], F32, tag="pa", name=f"pa{j}")
                       for j in range(2)]
                for k in range(KC):
                    for j in range(2):
                        n = np_ * 2 + j
                        nc.tensor.matmul(
                            pts[j][:],
                            lhsT_blk[:, k, mt * P:(mt + 1) * P],
                            w_T[:, k, n * 512:(n + 1) * 512],
                            start=(k == 0), stop=(k == KC - 1),
                        )
                for j in range(2):
                    n = np_ * 2 + j
                    ev = ev_pool.tile([P, 512], ADT, tag="aev")
                    nc.vector.tensor_add(ev[:], pts[j][:],
                                         bias128[:, n * 512:(n + 1) * 512])
                    nc.sync.dma_start(
                        a_dram_flat[row0:row0 + P, n * 512:(n + 1) * 512], ev[:]
                    )


def _emit_recurrence(nc, T, *, a_dram, w_T8, hT8, hTb, c_t, a_pool,
                     gact_pool, hpool, psum_g, psum_tr, misc_pool,
                     h1T_dram=None, dec=None, out_ap=None, ident=None,
                     identh=None, xw=None):
    """T sequential LSTM-cell steps for one layer.

    hT8: fp8 ring [128, KPF, 2, 8, BPC] (k-pair packed, DoubleRow lhsT);
    step t writes slot t%8, reads slot (t-1)%8.  hTb: optional bf16 ring
    [128, KC, 8, BPC] kept in parallel for the h1T flush / A2 path.
    c_t: [BPC, H] fp32 persistent cell state.
    Gate preactivation = PSUM preload (a_t + bias) + h @ W_hh^T (fp8
    DoubleRow: 4 matmuls per 512-wide gate half).
    Weight gate order along G: i, f, g, o.
    """
    a_tiles = {}

    def load_a(t):
        if a_dram is None:
            return
        a1t = a_pool.tile([BPC, G], ADT, tag="a1t", name=f"a1t_{t % 4}")
        nc.sync.dma_start(a1t[:], a_dram[t])
        a_tiles[t] = a1t

    load_a(0)
    pg_next = {}

    for t in range(T):
        s_r = (t + 7) % 8
        s_w = t % 8
        if t + 1 < T:
            load_a(t + 1)
        a1t = a_tiles.get(t)

        acts = {}

        def preload_pe(g_idx, pg, at, tt=None):
            # layer 1 (xw): a_t computed on the fly as x_t @ W_comb^T with
            # the bias riding as contraction row IN (ones row in xT) — no
            # A-scratch, no DMA. Layer 2: identity matmul injects a2_t.
            for n2 in range(2):
                n = g_idx * 2 + n2
                if xw is not None:
                    xT, wcT = xw
                    nc.tensor.matmul(
                        pg[:, n2 * 512:(n2 + 1) * 512],
                        xT[:, tt * BPC:(tt + 1) * BPC],
                        wcT[:, n * 512:(n + 1) * 512],
                        start=True, stop=False, skip_group_check=True,
                    )
                else:
                    nc.tensor.matmul(
                        pg[:, n2 * 512:(n2 + 1) * 512],
                        identh[:BPC, :BPC],
                        at[:, n * 512:(n + 1) * 512],
                        start=True, stop=False, skip_group_check=True,
                    )

        def mm_gate(g_idx, pg, n2_outer=False, preloaded=False):
            if not preloaded:
                preload_pe(g_idx, pg, a1t, tt=t)
            loops = ([(n2, kp) for n2 in (1, 0) for kp in range(KPF)] if n2_outer
                     else [(n2, kp) for kp in range(KPF) for n2 in range(2)])
            for n2, kp in loops:
                n = g_idx * 2 + n2
                nc.tensor.matmul(
                    pg[:, n2 * 512:(n2 + 1) * 512],
                    hT8[:, kp, :, s_r, :],
                    w_T8[:, kp, :, n * 512:(n + 1) * 512],
                    start=False, stop=(kp == KPF - 1),
                    perf_mode=mybir.MatmulPerfMode.DoubleRow,
                    skip_group_check=True,
                )

        def do_gate(g_idx, func, tag):
            pg = pg_next.pop(g_idx, None)
            if pg is None:
                pg = psum_g.tile([BPC, H], F32, tag="pg", name=f"pg{g_idx}")
                mm_gate(g_idx, pg)
            else:
                mm_gate(g_idx, pg, preloaded=True)
            at = gact_pool.tile([BPC, H], HDT, tag="gact", name=tag)
            nc.scalar.activation(at[:], pg[:], func)
            acts[g_idx] = at

        HF = 512  # half of H, processed separately to shorten the serial tail
        do_gate(0, AF.Sigmoid, "act_i")        # input gate
        do_gate(2, AF.Tanh, "act_g")           # candidate
        tmp = gact_pool.tile([BPC, H], HDT, tag="gact", name="tmp")
        nc.vector.tensor_mul(tmp[:], acts[0][:], acts[2][:])

        pg_f = psum_g.tile([BPC, H], F32, tag="pg", name="pg_f")
        mm_gate(1, pg_f, n2_outer=True)
        act_f = gact_pool.tile([BPC, H], HDT, tag="gact", name="act_f")
        tanh_c = gact_pool.tile([BPC, H], HDT, tag="gact", name="tanh_c")
        for hh in (1, 0):
            sl = slice(hh * HF, (hh + 1) * HF)
            nc.scalar.activation(act_f[:, sl], pg_f[:, sl], AF.Sigmoid)
            nc.vector.tensor_mul(c_t[:, sl], c_t[:, sl], act_f[:, sl])
            nc.vector.tensor_add(c_t[:, sl], c_t[:, sl], tmp[:, sl])
            nc.scalar.activation(tanh_c[:, sl], c_t[:, sl], AF.Tanh)

        # keepalive: the PE drops to the 1.2GHz p-state during the tail idle
        # and takes ~10 matmuls to recover; a tiny matmul keeps it hot
        ka1 = psum_tr.tile([1, 256], F32, tag="htr", name="ka1")
        nc.tensor.matmul(ka1[:], identh[:BPC, 0:1], tanh_c[:, 0:256],
                         start=True, stop=True)

        # output gate + h + h^T, in halves so hT chunks stream out early
        pg_o = psum_g.tile([BPC, H], F32, tag="pg", name="pg_o")
        mm_gate(3, pg_o, n2_outer=True)
        act_o = gact_pool.tile([BPC, H], HDT, tag="gact", name="act_o")
        h_t = hpool.tile([BPC, H], HDT, tag="h_t")
        for hh in (1, 0):
            sl = slice(hh * HF, (hh + 1) * HF)
            nc.scalar.activation(act_o[:, sl], pg_o[:, sl], AF.Sigmoid)
            nc.vector.tensor_mul(h_t[:, sl], act_o[:, sl], tanh_c[:, sl])
        ka2 = psum_tr.tile([1, 256], F32, tag="htr", name="ka2")
        nc.tensor.matmul(ka2[:], identh[:BPC, 0:1], tanh_c[:, 256:512],
                         start=True, stop=True)

        # gate-i preload of step t+1 rides before the transposes: the PE
        # does it while waiting on h_t, and the post-transpose restart goes
        # straight into DoubleRow matmuls
        if t + 1 < T:
            pg_i = psum_g.tile([BPC, H], F32, tag="pg", name="pg0")
            preload_pe(0, pg_i, a_tiles.get(t + 1), tt=t + 1)
            pg_next[0] = pg_i

        # reversed: chunk 0 (needed first by next step) lands last, so the
        # scheduler cannot interleave next-step matmuls with the transposes
        for k in range(KC - 1, -1, -1):
            pt = psum_tr.tile([P, BPC], HDT, tag="htr")
            nc.tensor.transpose(pt[:], h_t[:, k * P:(k + 1) * P], identh[:BPC, :BPC])
            if hTb is not None:
                # critical fp8 ring via vector; bf16 flush/A2 ring via the
                # scalar engine (both read the transpose PSUM in parallel)
                nc.vector.tensor_copy(hT8[:, k // 2, k % 2, s_w, :], pt[:])
                nc.scalar.activation(hTb[:, k, s_w, :], pt[:], AF.Copy)
            else:
                nc.vector.tensor_copy(hT8[:, k // 2, k % 2, s_w, :], pt[:])
                if dec is not None and t == T - 1:
                    # bf16 copy of the final h2^T for a full-precision decode
                    nc.scalar.activation(dec[3][:, k, :], pt[:], AF.Copy)

        if h1T_dram is not None and (s_w == 7 or t == T - 1):
            # flush the ring (contiguous runs per partition)
            nslots = s_w + 1
            col0 = (t // 8) * 8 * BPC
            nc.sync.dma_start(
                h1T_dram.rearrange("(c p) n -> p c n", p=P)[:, :, col0:col0 + nslots * BPC],
                hTb[:, :, 0:nslots, :],
            )

        if dec is not None and t == T - 1:
            decWT, decb_sb, ones_bpc, hT_last = dec
            pd = psum_g.tile([1, BPC], F32, tag="pg", name="pdec")
            for k in range(KC):
                nc.tensor.matmul(pd[:], decWT[:, k:k + 1],
                                 hT_last[:, k, :],
                                 start=(k == 0), stop=False)
            nc.tensor.matmul(pd[:], decb_sb[:], ones_bpc[:],
                             start=False, stop=True)
            osb = misc_pool.tile([1, BPC], F32, tag="osb")
            nc.vector.tensor_copy(osb[:], pd[:])
            nc.sync.dma_start(out_ap.rearrange("b o -> o b"), osb[:])


def build(T=100):
    nc = bacc.Bacc("TRN2", target_bir_lowering=False, debug=False,
                   num_devices=N_CORES)

    x = nc.dram_tensor("x", [BPC, T, IN], F32, kind="ExternalInput").ap()
    enc_W = nc.dram_tensor("enc_W", [H, IN], F32, kind="ExternalInput").ap()
    enc_b = nc.dram_tensor("enc_b", [H], F32, kind="ExternalInput").ap()
    W_ih1 = nc.dram_tensor("W_ih1", [G, H], F32, kind="ExternalInput").ap()
    W_hh1 = nc.dram_tensor("W_hh1", [G, H], F32, kind="ExternalInput").ap()
    b_ih1 = nc.dram_tensor("b_ih1", [G], F32, kind="ExternalInput").ap()
    b_hh1 = nc.dram_tensor("b_hh1", [G], F32, kind="ExternalInput").ap()
    W_ih2 = nc.dram_tensor("W_ih2", [G, H], F32, kind="ExternalInput").ap()
    W_hh2 = nc.dram_tensor("W_hh2", [G, H], F32, kind="ExternalInput").ap()
    b_ih2 = nc.dram_tensor("b_ih2", [G], F32, kind="ExternalInput").ap()
    b_hh2 = nc.dram_tensor("b_hh2", [G], F32, kind="ExternalInput").ap()
    dec_W = nc.dram_tensor("dec_W", [1, H], F32, kind="ExternalInput").ap()
    dec_b = nc.dram_tensor("dec_b", [1], F32, kind="ExternalInput").ap()
    out = nc.dram_tensor("out", [BPC, 1], F32, kind="ExternalOutput").ap()

    A1 = nc.dram_tensor("A1_scratch", [T, BPC, G], ADT).ap()
    A2 = nc.dram_tensor("A2_scratch", [T, BPC, G], ADT).ap()
    h1T = nc.dram_tensor("h1T_scratch", [H, T * BPC], HDT).ap()
    A1_flat = A1.rearrange("t b g -> (t b) g")
    A2_flat = A2.rearrange("t b g -> (t b) g")

    with tile.TileContext(nc) as tc:
        with tc.tile_pool(name="persist", bufs=1) as persist, \
             tc.tile_pool(name="state", bufs=1) as state, \
             tc.tile_pool(name="misc", bufs=1) as misc:

            ident = persist.tile([P, P], F32, tag="ident")
            make_identity(nc, ident[:])
            identr = persist.tile([P, P], F32R, tag="identr")
            nc.vector.tensor_copy(identr[:], ident[:])
            identh = persist.tile([P, P], HDT, tag="identh")
            nc.vector.tensor_copy(identh[:], ident[:])
            ones1 = persist.tile([1, P], F32R, tag="ones1")
            nc.gpsimd.memset(ones1[:].bitcast(F32), 1.0)

            # persistent layer-1 input-side operands: row IN of xT is all
            # ones and row IN of W_combT is the gate-bias row, so the K=35
            # preload matmul x_t @ W_comb^T lands A1+bias in one pass
            W_combT = persist.tile([IN + 1, G], F32R, tag="W_combT")
            xT = persist.tile([IN + 1, T * BPC], F32R, tag="xT")
            bias128_2 = persist.tile([P, G], ADT, tag="bias128_2")

            def bcast_bias_row(brow, dst, ps_pool):
                """dst[p, n*512:(n+1)*512] = brow[0, n*512:...] for all p."""
                for n in range(8):
                    sl = slice(n * 512, (n + 1) * 512)
                    pb2 = ps_pool.tile([P, 512], F32, tag="pbb")
                    nc.tensor.matmul(pb2[:], ones1[:], brow[:, sl],
                                     start=True, stop=True)
                    nc.vector.tensor_copy(dst[:, sl], pb2[:])

            # ============ Phase E: xT [IN+1, T*BPC] (row IN = ones) ======
            if True:
                with nc.named_scope("phaseE"):
                    with tc.tile_pool(name="e_sb", bufs=3) as e_sb, \
                         tc.tile_pool(name="e_ps", bufs=3, space="PSUM") as e_ps:
                        onesrow = e_sb.tile([1, T * BPC], F32R, tag="onesrow")
                        nc.gpsimd.memset(onesrow[:].bitcast(F32), 1.0)
                        # DMA (not an engine op) may target the unaligned
                        # partition offset IN=34
                        nc.sync.dma_start(xT[IN:IN + 1, :], onesrow[:])
                        xr = x.rearrange("b t f -> t b f")
                        for m in range((T * BPC) // P):
                            xt_ = e_sb.tile([P, IN], F32R, tag="xtile")
                            nc.sync.dma_start(xt_[:BPC, :], xr[2 * m].bitcast(F32R))
                            nc.sync.dma_start(xt_[BPC:, :], xr[2 * m + 1].bitcast(F32R))
                            pt = e_ps.tile([IN, P], F32R, tag="xtr")
                            nc.tensor.transpose(pt[:], xt_[:], identr[:])
                            nc.vector.tensor_copy(xT[0:IN, m * P:(m + 1) * P], pt[:])

                # ============ W_combT build (incremental, no 16MB W_ih1^T) ====
                # encwb [128, KC, IN+1]: cols 0..IN-1 = enc_W chunk rows,
                # col IN = enc_b chunk. A single lhsT gives both W_combT rows
                # and the enc_b @ W_ih1^T bias row in one PSUM pass.
                with nc.named_scope("build_Wcomb"):
                    with tc.tile_pool(name="wc_sb", bufs=1) as wc_sb, \
                         tc.tile_pool(name="wc_row", bufs=3) as wc_row, \
                         tc.tile_pool(name="wc_st", bufs=2) as wc_st, \
                         tc.tile_pool(name="wc_ps", bufs=2, space="PSUM") as wc_ps, \
                         tc.tile_pool(name="wc_ps2", bufs=1, space="PSUM") as wc_ps2:
                        encwb = wc_sb.tile([P, KC, IN], F32R, tag="encwb")
                        nc.sync.dma_start(
                            encwb[:],
                            enc_W.rearrange("(c p) f -> p c f", p=P).bitcast(F32R))
                        encb_k = wc_sb.tile([P, KC], F32R, tag="encb_k")
                        nc.sync.dma_start(
                            encb_k[:],
                            enc_b.rearrange("(c p) -> p c", p=P).bitcast(F32R))
                        brow1 = wc_sb.tile([1, G], F32R, tag="brow1")
                        bi1 = wc_sb.tile([1, G], F32, tag="bi1")
                        nc.sync.dma_start(bi1[:], b_ih1[None, :])
                        bh1 = wc_sb.tile([1, G], F32, tag="bh1")
                        nc.sync.dma_start(bh1[:], b_hh1[None, :])
                        # groups of 4 row-chunks = 512 G columns
                        for grp in range(G // 512):
                            wstage = wc_st.tile([P, KC, 512], F32R, tag="wstage")
                            for rr in range(4):
                                r = grp * 4 + rr
                                wt = wc_row.tile([P, H], F32R, tag="wcrow")
                                nc.sync.dma_start(
                                    wt[:], W_ih1[r * P:(r + 1) * P, :].bitcast(F32R))
                                for c in range(KC):
                                    ptr = wc_ps.tile([P, P], F32R, tag="wctr")
                                    nc.tensor.transpose(
                                        ptr[:], wt[:, c * P:(c + 1) * P], identr[:])
                                    if c % 2 == 0:
                                        nc.vector.tensor_copy(
                                            wstage[:, c, rr * P:(rr + 1) * P], ptr[:])
                                    else:
                                        nc.scalar.activation(
                                            wstage[:, c, rr * P:(rr + 1) * P],
                                            ptr[:], AF.Copy)
                            pb = wc_ps2.tile([IN, 512], F32, tag="wcpb")
                            pbias = wc_ps2.tile([1, 512], F32, tag="wcpbias")
                            for k in range(KC):
                                nc.tensor.matmul(pb[:], encwb[:, k, :],
                                                 wstage[:, k, :],
                                                 start=(k == 0), stop=(k == KC - 1))
                            for k in range(KC):
                                nc.tensor.matmul(pbias[:], encb_k[:, k:k + 1],
                                                 wstage[:, k, :],
                                                 start=(k == 0), stop=(k == KC - 1))
                            sl = slice(grp * 512, (grp + 1) * 512)
                            nc.vector.tensor_copy(W_combT[0:IN, sl], pb[:])
                            # bias row = enc_b@W^T + b_ih1 + b_hh1
                            nc.vector.tensor_add(brow1[:, sl], pbias[:],
                                                 bi1[:, sl])
                            nc.gpsimd.tensor_add(brow1[:, sl], brow1[:, sl],
                                                 bh1[:, sl])
                        # bias row rides as contraction row IN (DMA can hit
                        # the unaligned partition offset)
                        nc.sync.dma_start(W_combT[IN:IN + 1, :], brow1[:])

            # ============ W_hh1^T build + Phase R1 ============
            with tc.tile_pool(name="wpool", bufs=1) as wpool:
                w_T1 = wpool.tile([P, KPF, 2, G], FP8, tag="W8")
                with nc.named_scope("build_Whh1T"):
                    with tc.tile_pool(name="wrow1", bufs=3) as wrow, \
                         tc.tile_pool(name="wtr_ps1", bufs=3, space="PSUM") as wtr_ps:
                        _build_weight_T8(nc, W_hh1, w_T1, identr, wrow, wtr_ps)

                hT8 = state.tile([P, KPF, 2, 8, BPC], FP8, tag="hT8_ring")
                hTb = state.tile([P, KC, 8, BPC], HDT, tag="hTb_ring")
                c_t = state.tile([BPC, H], F32, tag="c_t")
                nc.gpsimd.memset(hT8[:].bitcast(mybir.dt.uint8), 0.0)
                nc.gpsimd.memset(hTb[:].bitcast(mybir.dt.uint16), 0.0)
                nc.gpsimd.memset(c_t[:], 0.0)

                with nc.named_scope("phaseR1"):
                    with tc.tile_pool(name="r1_a", bufs=2) as a_pool, \
                         tc.tile_pool(name="r1_g", bufs=4) as gact_pool, \
                         tc.tile_pool(name="r1_h", bufs=1) as hpool, \
                         tc.tile_pool(name="r1_pg", bufs=3, space="PSUM") as psum_g, \
                         tc.tile_pool(name="r1_ptr", bufs=2, space="PSUM") as psum_tr:
                        _emit_recurrence(nc, T, a_dram=None, xw=(xT, W_combT),
                                         w_T8=w_T1, hT8=hT8, hTb=hTb, c_t=c_t,
                                         a_pool=a_pool, gact_pool=gact_pool,
                                         hpool=hpool, psum_g=psum_g,
                                         psum_tr=psum_tr, misc_pool=misc,
                                         h1T_dram=h1T, ident=ident,
                                         identh=identh)

                # ============ W_ih2^T build + Phase A2 ============
                w_T2 = wpool.tile([P, KC, G], WDT, tag="W")
                with nc.named_scope("build_Wih2T"):
                    with tc.tile_pool(name="wrow2", bufs=3) as wrow, \
                         tc.tile_pool(name="wtr_ps2", bufs=3, space="PSUM") as wtr_ps:
                        _build_weight_T(nc, W_ih2, w_T2, identr, wrow, wtr_ps)

                with nc.named_scope("phaseA2"):
                    # layer-2 bias row: b_ih2 + b_hh2 broadcast
                    with tc.tile_pool(name="b2_sb", bufs=2) as b2_sb, \
                         tc.tile_pool(name="b2_ps", bufs=2, space="PSUM") as b2_ps:
                        for n in range(8):
                            slb = slice(n * 512, (n + 1) * 512)
                            bi2 = b2_sb.tile([1, 512], F32, tag="bi2")
                            nc.sync.dma_start(bi2[:], b_ih2[None, slb])
                            bh2 = b2_sb.tile([1, 512], F32, tag="bh2")
                            nc.sync.dma_start(bh2[:], b_hh2[None, slb])
                            brow2 = b2_sb.tile([1, 512], F32R, tag="brow2")
                            nc.vector.tensor_add(brow2[:], bi2[:], bh2[:])
                            pb2 = b2_ps.tile([P, 512], F32, tag="pbb")
                            nc.tensor.matmul(pb2[:], ones1[:], brow2[:],
                                             start=True, stop=True)
                            nc.vector.tensor_copy(bias128_2[:, slb], pb2[:])

                    h1T_r = h1T.rearrange("(c p) n -> p c n", p=P)
                    with tc.tile_pool(name="h1_sb", bufs=2) as h1_sb, \
                         tc.tile_pool(name="a_ps2", bufs=4, space="PSUM") as a_ps, \
                         tc.tile_pool(name="a_ev2", bufs=2) as a_ev:

                        def h1_blk(mb):
                            blk = h1_sb.tile([P, KC, 256], HDT, tag="h1blk")
                            nc.sync.dma_start(
                                blk[:], h1T_r[:, :, mb * 256:(mb + 1) * 256])
                            return blk

                        _emit_A2_phase(nc, T, w_T=w_T2, bias128=bias128_2,
                                       lhs_blk_fn=h1_blk,
                                       a_dram_flat=A2_flat,
                                       psum_a=a_ps, ev_pool=a_ev)

                # ============ W_hh2^T build + Phase R2 (+decode) ============
                w_T3 = wpool.tile([P, KPF, 2, G], FP8, tag="W8")
                with nc.named_scope("build_Whh2T"):
                    with tc.tile_pool(name="wrow3", bufs=3) as wrow, \
                         tc.tile_pool(name="wtr_ps3", bufs=3, space="PSUM") as wtr_ps:
                        _build_weight_T8(nc, W_hh2, w_T3, identr, wrow, wtr_ps)

                nc.gpsimd.memset(hT8[:].bitcast(mybir.dt.uint8), 0.0)
                nc.gpsimd.memset(c_t[:], 0.0)

                decWT_f = misc.tile([P, KC], F32, tag="decWT_f")
                nc.sync.dma_start(decWT_f[:], dec_W.rearrange("o (c p) -> p (c o)", p=P))
                decWT = misc.tile([P, KC], HDT, tag="decWT")
                nc.vector.tensor_copy(decWT[:], decWT_f[:])
                decb_f = misc.tile([1, 1], F32, tag="decb_f")
                nc.sync.dma_start(decb_f[:], dec_b[None, :])
                decb_sb = misc.tile([1, 1], HDT, tag="decb")
                nc.vector.tensor_copy(decb_sb[:], decb_f[:])
                ones_f = misc.tile([1, BPC], F32, tag="ones_f")
                nc.gpsimd.memset(ones_f[:], 1.0)
                ones_bpc = misc.tile([1, BPC], HDT, tag="ones_bpc")
                hT_last = misc.tile([P, KC, BPC], HDT, tag="hT_last")
                nc.vector.tensor_copy(ones_bpc[:], ones_f[:])

                with nc.named_scope("phaseR2"):
                    with tc.tile_pool(name="r2_a", bufs=2) as a_pool, \
                         tc.tile_pool(name="r2_g", bufs=4) as gact_pool, \
                         tc.tile_pool(name="r2_h", bufs=1) as hpool, \
                         tc.tile_pool(name="r2_pg", bufs=3, space="PSUM") as psum_g, \
                         tc.tile_pool(name="r2_ptr", bufs=2, space="PSUM") as psum_tr:
                        _emit_recurrence(nc, T, a_dram=A2,
                                         w_T8=w_T3, hT8=hT8, hTb=None, c_t=c_t,
                                         a_pool=a_pool, gact_pool=gact_pool,
                                         hpool=hpool, psum_g=psum_g,
                                         psum_tr=psum_tr, misc_pool=misc,
                                         h1T_dram=None,
                                         dec=(decWT, decb_sb, ones_bpc, hT_last),
                                         out_ap=out, ident=ident, identh=identh)

    nc.compile()
    return nc


_cached_nc = None
_cached_fn = None  # (jitted shard_map fn, in_names, out_names, out_shapes, zeros)


def _build_jitted(nc):
    """Same lowering as bass2jax.run_bass_via_pjrt, but the jitted
    executable is cached so repeat kernel() calls skip recompilation."""
    import jax
    from jax.sharding import Mesh, PartitionSpec
    from jax.experimental.shard_map import shard_map
    from concourse import bass2jax, mybir as _mybir

    bass2jax.install_neuronx_cc_hook()
    partition_name = nc.partition_id_tensor.name if nc.partition_id_tensor else None
    in_names, out_names, out_avals, zero_outs = [], [], [], []
    for alloc in nc.m.functions[0].allocations:
        if not isinstance(alloc, _mybir.MemoryLocationSet):
            continue
        name = alloc.memorylocations[0].name
        if alloc.kind == "ExternalInput":
            if name != partition_name:
                in_names.append(name)
        elif alloc.kind == "ExternalOutput":
            shape = tuple(alloc.tensor_shape)
            dtype = _mybir.dt.np(alloc.dtype)
            out_names.append(name)
            out_avals.append(jax.core.ShapedArray(shape, dtype))
            zero_outs.append(np.zeros(shape, dtype))
    n_params = len(in_names)
    n_outs = len(out_avals)
    all_in_names = list(in_names) + list(out_names)
    if partition_name is not None:
        all_in_names.append(partition_name)
    donate = tuple(range(n_params, n_params + n_outs))

    def _body(*args):
        operands = list(args)
        if partition_name is not None:
            operands.append(bass2jax.partition_id_tensor())
        outs = bass2jax._bass_exec_p.bind(
            *operands,
            out_avals=tuple(out_avals),
            in_names=tuple(all_in_names),
            out_names=tuple(out_names),
            lowering_input_output_aliases=(),
            sim_require_finite=True,
            sim_require_nnan=True,
            nc=nc,
        )
        return tuple(outs)

    devices = jax.devices()[:N_CORES]
    mesh = Mesh(np.asarray(devices), ("core",))
    in_specs = (PartitionSpec("core"),) * (n_params + n_outs)
    out_specs = (PartitionSpec("core"),) * n_outs
    fn = jax.jit(
        shard_map(_body, mesh=mesh, in_specs=in_specs, out_specs=out_specs,
                  check_rep=False),
        donate_argnums=donate, keep_unused=True,
    )
    out_shapes = [a.shape for a in out_avals]
    return fn, in_names, out_names, out_shapes, zero_outs


_dev_cache = {}  # name -> (digest, device_array)


def _to_device(name, arr):
    """Replicate-concat a weight to all cores and keep it on device across
    calls (keyed by content hash) so repeat kernel() calls only ship x."""
    import hashlib
    import jax
    d = hashlib.blake2b(arr.tobytes(), digest_size=16).digest()
    hit = _dev_cache.get(name)
    if hit is not None and hit[0] == d:
        return hit[1]
    conc = np.concatenate([arr] * N_CORES, axis=0)
    darr = jax.device_put(conc)
    _dev_cache[name] = (d, darr)
    return darr


def kernel(**inputs):
    global _cached_nc, _cached_fn
    if _cached_nc is None:
        _cached_nc = build(100)
        _cached_fn = _build_jitted(_cached_nc)
    fn, in_names, out_names, out_shapes, zero_outs = _cached_fn
    ins = {k: np.ascontiguousarray(np.asarray(v, dtype=np.float32))
           for k, v in inputs.items()}
    concat_in = []
    for name in in_names:
        if name == "x":
            concat_in.append(ins["x"])  # already [512, T, IN]; axis0 shards
        else:
            concat_in.append(_to_device(name, ins[name]))
    i = out_names.index("out")
    last_err = None
    for attempt in range(3):
        try:
            concat_zeros = [np.zeros((N_CORES * z.shape[0], *z.shape[1:]), z.dtype)
                            for z in zero_outs]
            out_arrs = fn(*concat_in, *concat_zeros)
            outp = np.asarray(out_arrs[i]).reshape(B, 1)
            return outp.astype(np.float32)
        except Exception as e:  # transient NRT_EXEC_UNIT_UNRECOVERABLE etc.
            last_err = e
            _dev_cache.clear()
            concat_in = []
            for name in in_names:
                if name == "x":
                    concat_in.append(ins["x"])
                else:
                    concat_in.append(_to_device(name, ins[name]))
    raise last_err
